# revision 31
# baseline (speedup 1.0000x reference)
"""Trainium2 Bass kernel for nn_DecoderLayer_60060822667509.

Data-parallel over the 4096 tokens (512/core on 8 cores). Routing
(host-side argmax on small logits, mirroring the reference's .item()
syncs) is computed from the actual inputs at call time and a
specialized Bass/Tile program is emitted for the selected DAG.

Design (v2):
- Activations feature-major on-chip ([128 features, NFC chunks, TOK
  tokens]); matmul outputs feed the next matmul's moving operand with
  no transposes.
- LayerNorms are materialized ONCE per source tensor (stats via
  PE ones-matmuls, apply via two DVE passes); every matmul is then a
  plain matmul on a unit-LN tensor with selection/activation scalars
  folded into the bf16 weights host-side.
- All weights are uploaded and DMA'd to SBUF at kernel start in use
  order; nothing is ever spilled to DRAM.
- Edge matmuls are emitted as soon as their source tensor exists
  (lookahead over the route DAG), so the PE queue never head-of-line
  blocks on LN statistics of the node being assembled.
- The final sum (unprocessed nodes) is accumulated in-place in f32 as
  contributions become ready, several directly from PSUM.
"""
import numpy as np
import ml_dtypes
from contextlib import ExitStack

import concourse.bass as bass
import concourse.tile as tile
from concourse import mybir
from concourse.bass import ts
from concourse.bass_utils import run_bass_kernel_spmd
from concourse.masks import make_identity

F32 = mybir.dt.float32
BF16 = mybir.dt.bfloat16
AF = mybir.ActivationFunctionType
ALU = mybir.AluOpType

ISIZE = 512
NNOD = 8
MAXP = 5
TAU = 1.0
EPS = 1e-6
B = 4
SLEN = 1024
NCORE = 8
TOK = (B * SLEN) // NCORE  # 512 tokens per core
NFC = ISIZE // 128         # 4 feature chunks
NTT = TOK // 128           # 4 token tiles


# ---------------------------------------------------------------------------
# Host-side routing (mirrors reference._routing exactly)
# ---------------------------------------------------------------------------

def _qmask(nsrc):
    m = np.zeros((nsrc, 5), bool)
    m[0, :] = True
    return m.reshape(-1)


def _routing(node_p, edge_p):
    node_p = np.asarray(node_p)
    edge_p = np.asarray(edge_p)
    routes, lind = [], 0
    for c in range(NNOD):
        nsrc = min(c + 2, MAXP)
        snode = c - nsrc
        ep = edge_p[:, lind:lind + nsrc, :].reshape(3, -1)
        qm = _qmask(nsrc)
        nact = int(np.argmax(node_p[c]))
        qsel = int(np.argmax(np.where(qm, -np.inf, ep[0])))
        r = dict(lind=lind, nsrc=nsrc, snode=snode, act=nact, q=qsel, k=None,
                 v=None, ktype=None, km=None, vmode=None)
        if nact < 7:
            km = qm if nact > 0 else None
            kl = ep[1] if km is None else np.where(km, -np.inf, ep[1])
            r['k'] = int(np.argmax(kl))
            r['km'] = km
            r['ktype'] = -2 if r['k'] // 5 == 0 else -1
            if nact < 5:
                if nact == 0 and r['ktype'] == -2:
                    r['v'] = int(np.argmax(ep[2][:5]))
                    r['vmode'] = 'first5'
                else:
                    vl = ep[2] if km is None else np.where(km, -np.inf, ep[2])
                    r['v'] = int(np.argmax(vl))
                    r['vmode'] = 'full'
        routes.append(r)
        lind += nsrc
    return routes


def _softmax_np(x):
    x = np.asarray(x, np.float64)
    e = np.exp(x - x.max())
    return e / e.sum()


def _selw_np(logits, mask, sel):
    logits = np.asarray(logits, np.float64)
    if mask is not None:
        logits = np.where(np.asarray(mask), -np.inf, logits)
    return float(_softmax_np(logits / TAU)[sel])


# ---------------------------------------------------------------------------
# TileContext with a walrus-compatible tail drain: this compiler build
# rejects sem waits on SP Drain/NoOp (TPB_CTRL has no wait slots), so
# emit the end-of-kernel waits as standalone wait_ge instructions.
# ---------------------------------------------------------------------------

class FixedTileContext(tile.TileContext):
    def _drain_and_barrier(self, tick_clock, wait_clock):
        nc = self.nc
        clock = list(tick_clock.global_clock)
        for p, sem in sorted(self.sems.allocated().items()):
            c = clock[p]
            if c > 0:
                mult = 16 if sem.name.startswith("DMA") else 1
                nc.sync.wait_ge(sem, c * mult)
        nc.sync.drain()
        nc.all_engine_barrier()
        popped = nc._tile_sem_poison_stack.pop()
        assert popped is self._sem_poison
        nc.clear_and_free_semaphores(list(self.sems.allocated().values()))
        nc.all_engine_barrier()


# ---------------------------------------------------------------------------
# Walrus-compat post-pass: at most one sync wait per engine instruction
# (none on SP control ops). Hoist excess waits onto standalone
# InstEventSemaphore instructions inserted before.
# ---------------------------------------------------------------------------

_NO_HOIST = ("InstEventSemaphore", "InstAllEngineBarrier",
             "InstCollectiveCompute")


def _hoist_excess_waits(nc):
    n = 0
    for f in nc.m.functions:
        for bb in f.blocks:
            out = []
            changed = False
            for inst in bb.instructions:
                tname = type(inst).__name__
                si = inst.sync_info
                if si is not None and tname not in _NO_HOIST:
                    waits = list(si.on_wait)
                    limit = 0 if tname in ("InstDrain", "InstNoOp") else 1
                    if len(waits) > limit:
                        for w in waits[:len(waits) - limit]:
                            n += 1
                            ni = mybir.InstEventSemaphore(
                                name=f"I-hoist{n}", ins=[], outs=[])
                            ni.engine = inst.engine
                            ni.sync_info = mybir.SyncInfo(on_wait=[w],
                                                          on_update=[])
                            out.append(ni)
                        si.on_wait = waits[len(waits) - limit:]
                        changed = True
                out.append(inst)
            if changed:
                bb.instructions = out
    return n


# ---------------------------------------------------------------------------
# Values: SBUF tensor [128, NFC, TOK] plus a symbolic host scalar.
# true value = mult * tensor. unit => tensor is a unit LayerNorm output.
# ---------------------------------------------------------------------------

class Val:
    def __init__(self, t, mult=1.0, unit=False):
        self.t = t
        self.mult = float(mult)
        self.unit = unit


class Builder:
    def __init__(self, nc, tc, ctx):
        self.nc = nc
        self.tc = tc
        self.uploads = {}
        self.n_tag = 0
        self.pool = ctx.enter_context(tc.tile_pool(name="act", bufs=1))
        self.ps_pool = ctx.enter_context(
            tc.tile_pool(name="ps", bufs=5, space="PSUM"))
        self.ps_bf = ctx.enter_context(
            tc.tile_pool(name="psb", bufs=1, space="PSUM"))
        self.ps_stat = ctx.enter_context(
            tc.tile_pool(name="pstat", bufs=2, space="PSUM"))
        self.ident_bf = self.pool.tile([128, 128], BF16, tag="idb")
        ih = self.upload("ident", np.eye(128).astype(ml_dtypes.bfloat16),
                         [128, 128], BF16)
        nc.sync.dma_start(self.ident_bf[:, :], ih[:, :])
        self.ones_bf = self.pool.tile([128, 1], BF16, tag="ones")
        nc.vector.memset(self.ones_bf, 1.0)
        self.ones_row_bf = self.pool.tile([1, 128], BF16, tag="onesr")
        nc.vector.memset(self.ones_row_bf, 1.0)
        self._cc_cache = {}
        self.stats_cache = {}   # id(tensor) -> (rb_sb, mr_sb)
        self.ln_cache = {}      # id(tensor) -> Val (unit LN)
        # shared scratch (serial across stats/LN calls)
        self.sm_shared = self.pool.tile([1, 4 * TOK], F32, tag="smsh")
        self.rm_shared = self.pool.tile([1, 2 * TOK], BF16, tag="rmsh")
        self.x2_shared = self.pool.tile([128, NFC, TOK], BF16, tag="x2sh")
        self.lt_shared = self.pool.tile([128, NFC, TOK], BF16, tag="ltsh")
        self.rt_shared = self.pool.tile([128, NFC, TOK], BF16, tag="rtsh")

    def tag(self, kind="t"):
        self.n_tag += 1
        return f"{kind}{self.n_tag}"

    def sb(self, shape, dtype, kind="a"):
        tg = self.tag(kind)
        return self.pool.tile(list(shape), dtype, tag=tg, name=tg)

    def const_col(self, value, parts=1):
        key = (float(value), parts)
        if key not in self._cc_cache:
            t = self.pool.tile([parts, 1], F32, tag=self.tag("cc"))
            self.nc.vector.memset(t, float(value))
            self._cc_cache[key] = t
        return self._cc_cache[key]

    # -- host->device uploads -----------------------------------------------
    def upload(self, base, arrs, shape, dtype):
        name = f"{base}{len(self.uploads)}"
        if not isinstance(arrs, list):
            arrs = [arrs] * NCORE
        self.uploads[name] = [np.ascontiguousarray(a) for a in arrs]
        return self.nc.declare_dram_parameter(name, list(shape), dtype,
                                              isOutput=False)

    def upload_weight(self, w_np):
        """w_np [512, 512] (in, out) -> bf16 SBUF tile [128, NFC, 512]."""
        arr = np.ascontiguousarray(
            np.asarray(w_np, np.float32).reshape(NFC, 128, ISIZE)
            .transpose(1, 0, 2)).astype(ml_dtypes.bfloat16)
        hdl = self.upload("w", arr, [128, NFC, ISIZE], BF16)
        t = self.sb([128, NFC, ISIZE], BF16, kind="w")
        self.nc.sync.dma_start(t[:, :, :], hdl[:, :, :])
        return t

    def upload_bias(self, b_np):
        """b_np [512] -> SBUF [128, NFC] f32 (per-partition scalars)."""
        arr = np.ascontiguousarray(
            np.asarray(b_np, np.float32).reshape(NFC, 128).transpose(1, 0))
        hdl = self.upload("b", arr, [128, NFC], F32)
        t = self.sb([128, NFC], F32, kind="bias")
        self.nc.sync.dma_start(t[:, :], hdl[:, :])
        return t

    # -- input load ----------------------------------------------------------
    def load_input_fm(self, hdl):
        """DRAM [TOK, 512] bf16 token-major -> feature-major bf16 tensor."""
        nc = self.nc
        out = self.sb([128, NFC, TOK], BF16, kind="in")
        tok_tiles = []
        for tt in range(NTT):
            t = self.sb([128, ISIZE], BF16, kind="int")
            nc.sync.dma_start(t[:, :], hdl[ts(tt, 128), :])
            tok_tiles.append(t)
        for fc in range(NFC):
            ps = self.ps_bf.tile([128, TOK], BF16, tag="psb")
            for tt in range(NTT):
                nc.tensor.transpose(ps[:, ts(tt, 128)],
                                    tok_tiles[tt][:, ts(fc, 128)],
                                    self.ident_bf)
            nc.scalar.activation(out[:, fc, :], ps[:, :], AF.Identity)
        return Val(out, 1.0, False)

    # -- LayerNorm infra -----------------------------------------------------
    def ln_stats(self, val):
        """Per-token stats of the stored tensor: returns (rb_sb, mr_sb),
        both [128, TOK] bf16 broadcasts of rstd' and mean*rstd', such that
        LN(true) = tensor*rb - mr.  eps' = EPS / mult^2."""
        key = id(val.t)
        if key in self.stats_cache:
            return self.stats_cache[key]
        nc = self.nc
        x = val.t
        x2 = self.x2_shared
        m_ps = self.ps_stat.tile([1, TOK], F32, tag="st")
        s2_ps = self.ps_stat.tile([1, TOK], F32, tag="st")
        for kc in range(NFC):
            nc.tensor.matmul(m_ps[:, :], self.ones_bf[:, :], x[:, kc, :],
                             start=(kc == 0), stop=(kc == NFC - 1))
            nc.vector.tensor_mul(x2[:, kc, :], x[:, kc, :], x[:, kc, :])
            nc.tensor.matmul(s2_ps[:, :], self.ones_bf[:, :], x2[:, kc, :],
                             start=(kc == 0), stop=(kc == NFC - 1))
        # mean row (bf16, for the fused mean-correction matmul)
        m_bf = self.sb([1, TOK], BF16, kind="mb")
        nc.vector.tensor_scalar_mul(m_bf[:, :], m_ps[:, :], 1.0 / ISIZE)
        sm = self.sm_shared
        sv = sm[:, 0:TOK]
        nc.vector.scalar_tensor_tensor(sv, m_bf[:, :], -1.0, m_bf[:, :],
                                       op0=ALU.mult, op1=ALU.mult)  # -mean^2
        nc.vector.scalar_tensor_tensor(sv, s2_ps[:, :], 1.0 / ISIZE, sv,
                                       op0=ALU.mult, op1=ALU.add)   # var
        epsp = EPS / (val.mult * val.mult)
        r_bf = self.rm_shared
        nc.scalar.activation(sv, sv, AF.Ln, bias=self.const_col(epsp))
        nc.scalar.activation(r_bf[:, 0:TOK], sv, AF.Exp, scale=-0.5)
        rb_ps = self.ps_pool.tile([128, TOK], F32, tag="ps")
        nc.tensor.matmul(rb_ps[:, :], self.ones_row_bf[:, :],
                         r_bf[:, 0:TOK], start=True, stop=True)
        rb_sb = self.sb([128, TOK], BF16, kind="rb")
        nc.scalar.activation(rb_sb[:, :], rb_ps[:, :], AF.Identity)
        ent = dict(m_bf=m_bf, rb=rb_sb, mr=None)
        self.stats_cache[key] = ent
        return ent

    def ln_mr(self, val):
        """mr broadcast (mean*rstd, [128,TOK] bf16) for materializing."""
        ent = self.ln_stats(val)
        if ent['mr'] is None:
            nc = self.nc
            r_bf = self.rm_shared
            nc.vector.scalar_tensor_tensor(
                r_bf[:, TOK:2 * TOK], ent['m_bf'][:, :], 1.0,
                r_bf[:, 0:TOK], op0=ALU.mult, op1=ALU.mult)
            mr_ps = self.ps_pool.tile([128, TOK], F32, tag="ps")
            nc.tensor.matmul(mr_ps[:, :], self.ones_row_bf[:, :],
                             r_bf[:, TOK:2 * TOK], start=True, stop=True)
            mr_sb = self.sb([128, TOK], BF16, kind="mr")
            nc.scalar.activation(mr_sb[:, :], mr_ps[:, :], AF.Identity)
            ent['mr'] = mr_sb
        return ent

    def ln_of(self, val):
        """Materialized unit-LN of val (cached). Per-chunk two-pass apply:
        u = x*rb - mr."""
        if val.unit:
            kappa = 1.0 / np.sqrt(1.0 + EPS / (val.mult * val.mult))
            return Val(val.t, kappa, True)
        key = id(val.t)
        if key in self.ln_cache:
            return self.ln_cache[key]
        nc = self.nc
        ent = self.ln_mr(val)
        rb_sb, mr_sb = ent['rb'], ent['mr']
        u = self.sb([128, NFC, TOK], BF16, kind="ln")
        tmp = self.lt_shared
        for fc in range(NFC):
            nc.vector.tensor_mul(tmp[:, fc, :], val.t[:, fc, :], rb_sb[:, :])
            nc.vector.scalar_tensor_tensor(
                u[:, fc, :], mr_sb[:, :], -1.0, tmp[:, fc, :],
                op0=ALU.mult, op1=ALU.add)
        out = Val(u, 1.0, True)
        self.ln_cache[key] = out
        return out

    # -- matmul --------------------------------------------------------------
    def mm_site_ln(self, val, w_np, epilogue):
        """Fused-LN matmul: LN(val) @ w, running on the RAW tensor.
        Mean is subtracted inside PSUM via a K=1 matmul with the negated
        column sums of w; rstd is applied in the epilogue, which receives
        (mc, ps, rb)."""
        nc = self.nc
        ent = self.ln_stats(val)
        wbf = np.asarray(w_np, np.float32).astype(ml_dtypes.bfloat16)
        wt = self.upload_weight(wbf)
        wcs = np.ascontiguousarray(
            -wbf.astype(np.float32).sum(axis=0)[None, :]
        ).astype(ml_dtypes.bfloat16)
        hw = self.upload("wc", wcs, [1, ISIZE], BF16)
        wcs_t = self.sb([1, ISIZE], BF16, kind="wc")
        nc.sync.dma_start(wcs_t[:, :], hw[:, :])
        x = val.t
        for mc in range(NFC):
            ps = self.ps_pool.tile([128, TOK], F32, tag="ps")
            for kc in range(NFC):
                nc.tensor.matmul(ps[:, :], wt[:, kc, ts(mc, 128)],
                                 x[:, kc, :], start=(kc == 0), stop=False)
            nc.tensor.matmul(ps[:, :], wcs_t[0:1, ts(mc, 128)],
                             ent['m_bf'][:, :], start=False, stop=True)
            epilogue(mc, ps, ent['rb'])

    def mm_site(self, parts, epilogue):
        """sum_i parts[i] @ W_i accumulated per output chunk; epilogue(mc, ps)
        consumes each chunk's PSUM. parts: list of (tensor, W_np) with all
        scalars folded into W host-side."""
        nc = self.nc
        wts = [self.upload_weight(w) for _, w in parts]
        for mc in range(NFC):
            ps = self.ps_pool.tile([128, TOK], F32, tag="ps")
            n = len(parts) * NFC
            i = 0
            for wt, (x, _) in zip(wts, parts):
                for kc in range(NFC):
                    nc.tensor.matmul(ps[:, :], wt[:, kc, ts(mc, 128)],
                                     x[:, kc, :], start=(i == 0),
                                     stop=(i == n - 1))
                    i += 1
            epilogue(mc, ps)

    def relu_ps(self, out_ap, ps_ap, scale):
        self.nc.scalar.activation(out_ap, ps_ap, AF.Relu,
                                  scale=float(scale))

    def copy_ps(self, out_ap, ps_ap):
        self.nc.scalar.activation(out_ap, ps_ap, AF.Identity)

    def act_epilogue(self, func, out, scale=1.0, bias_t=None):
        """Returns an epilogue writing func(scale*ps + bias) into out."""
        nc = self.nc

        def epi(mc, ps):
            bias_ap = bias_t[:, mc:mc + 1] if bias_t is not None else 0.0
            nc.scalar.activation(out[:, mc, :], ps[:, :], func,
                                 bias=bias_ap, scale=float(scale))
        return epi


# ---------------------------------------------------------------------------
# Graph emission
# ---------------------------------------------------------------------------

def _emit_graph(bld, np_in, routes):
    nc = bld.nc
    eW = np.asarray(np_in['edge_W'], np.float64)
    eb = np.asarray(np_in['edge_b'], np.float64)
    eg = np.asarray(np_in['edge_g'], np.float64)
    ebe = np.asarray(np_in['edge_beta'], np.float64)
    nW = np.asarray(np_in['node_W'], np.float64)
    nb = np.asarray(np_in['node_b'], np.float64)
    ng = np.asarray(np_in['node_g'], np.float64)
    nbe = np.asarray(np_in['node_beta'], np.float64)
    node_p = np.asarray(np_in['node_p'], np.float64)
    edge_p = np.asarray(np_in['edge_p'], np.float64)

    for r in routes:
        assert r['act'] != 0, "attention routing not supported in v2 kernel"

    # ---- route analysis ----------------------------------------------------
    # edge list: one entry per (consumer c, which) with selection scalar.
    # uses[(c, which)] = dict(src, e, op, s)
    uses = {}
    processed = set()
    used_src = set()
    for c, r in enumerate(routes):
        lind, nsrc = r['lind'], r['nsrc']
        ep = edge_p[:, lind:lind + nsrc, :].reshape(3, -1)
        for which, sel in (('q', r['q']), ('k', r['k']), ('v', r['v'])):
            if sel is None:
                continue
            se, op = sel // 5, sel % 5
            src = -2 if se == 0 else r['snode'] + se
            logits = ep[{'q': 0, 'k': 1, 'v': 2}[which]]
            first5 = (which == 'v' and r['vmode'] == 'first5')
            if first5:
                logits = logits[:5]
            mask = _qmask(nsrc) if which == 'q' else r['km']
            if first5:
                mask = None
            s = _selw_np(logits, mask, sel)
            uses[(c, which)] = dict(src=src, e=lind + se, op=op, s=s)
            processed.add(src)
            used_src.add(src)

    # which sources need LN (feed op<=2 edges)
    needs_ln = {u['src'] for u in uses.values() if u['op'] <= 2}
    aw = {c: float(_softmax_np(node_p[c] / TAU)[routes[c]['act']])
          for c in range(NNOD)}

    # final-sum membership: nodes never consumed as a source
    rem_nodes = [i for i in range(NNOD) if i not in processed]

    # ---- value bookkeeping -------------------------------------------------
    outs = {}          # node idx -> Val
    raw_of = {}        # node idx -> pre-LN raw Val (for fused-LN consumers)
    edge_h = {}        # e -> Val  (raw h of LN-edge or linear edge, unscaled)
    edge_emitted = set()

    # acc: the final sum, accumulated in-place, f32, true scale
    acc = bld.sb([128, NFC, TOK], BF16, kind="acc")
    acc_started = [False]

    def acc_add_ps(mc, ps, scale=1.0):
        """acc[:, mc, :] += scale * ps   (or initialize)."""
        if not acc_started[0]:
            nc.scalar.activation(acc[:, mc, :], ps[:, :], AF.Identity,
                                 scale=float(scale))
        else:
            nc.vector.scalar_tensor_tensor(
                acc[:, mc, :], ps[:, :], float(scale), acc[:, mc, :],
                op0=ALU.mult, op1=ALU.add)

    def acc_add_full(x, scale):
        """acc += scale * x (full tile, SBUF tensor)."""
        assert acc_started[0]
        for fc in range(NFC):
            nc.vector.scalar_tensor_tensor(
                acc[:, fc, :], x[:, fc, :], float(scale), acc[:, fc, :],
                op0=ALU.mult, op1=ALU.add)

    def acc_mark_started():
        acc_started[0] = True

    # does this (c, which) use feed the final accumulator directly?
    # -> node c is in rem AND its act combines terms additively for this slot
    def direct_to_acc(c, which):
        if c not in rem_nodes:
            return False
        a = routes[c]['act']
        # act6: q + k ; act5: q + gelu(k@W1+b1) (q slot only)
        # act4: q*sig(k) + v (v slot only)
        return (a == 6) or (a == 5 and which == 'q') or \
               (a == 4 and which == 'v')

    # multiplier applied to node c's term for `which` inside the final sum
    def acc_scale(c, which):
        return aw[c] * uses[(c, which)]['s']

    # ---- edge emission -----------------------------------------------------
    def w_eff(u):
        """Effective weight for an edge use (LN affine folded; for op3 the
        source mult is folded by the caller)."""
        e, op = u['e'], u['op']
        if op <= 2:
            return eg[e][:, None] * eW[e]
        return eW[e]

    def b_eff(u):
        e, op = u['e'], u['op']
        if op <= 2:
            return ebe[e] @ eW[e] + eb[e]
        return eb[e]

    def edge_input_ready(u):
        """(val, fused) the edge's matmul streams, or None if not ready.
        For LN edges on a non-unit source (or one with a recorded raw
        tensor), the matmul fuses the LN on the raw tensor."""
        src = u['src']
        if src not in outs:
            return None
        v = outs[src]
        if u['op'] in (0, 1, 2):
            if src in raw_of:
                return (raw_of[src], True)
            if not v.unit:
                return (v, True)
            return (bld.ln_of(v), False)
        return (v, False)

    def emit_edge(c, which, u):
        """Emit matmul + epilogue for one (consumer, which) use."""
        op, e, s = u['op'], u['e'], u['s']
        src_v = outs[u['src']]
        if op == 4:
            return  # identity: no work
        ready = edge_input_ready(u)
        assert ready is not None
        lnv, fused = ready
        x, xmult = lnv.t, lnv.mult
        if fused:
            xmult = 1.0  # LN of the raw tensor is unit by construction
        # collect all uses sharing this e (same consumer & src by construct)
        forms = [(w2, u2) for (c2, w2), u2 in uses.items()
                 if c2 == c and u2['e'] == e and u2['op'] != 4]
        key = e
        if key in edge_emitted:
            return
        edge_emitted.add(key)
        ops = {u2['op'] for _, u2 in forms}
        w = w_eff(u) * xmult
        b = b_eff(u)
        has_b = bool(np.any(b))

        if fused and has_b:
            fused = False
            lnv2 = bld.ln_of(lnv)
            x, xmult = lnv2.t, lnv2.mult
            w = w_eff(u) * xmult

        if ops == {0} and len(forms) == 1:
            # relu-only: fold consumer scalar (>0) through relu; the bias is
            # pre-scaled so Relu(cs*ps + cs*b) = cs*relu(ps + b)
            if direct_to_acc(c, which):
                cs = acc_scale(c, which)
                bias_t = bld.upload_bias(np.asarray(b) * cs) if has_b else None
                if fused:
                    # relu(rb*ps)*cs = rb>0 -> cs*rb*relu(ps)
                    def epi(mc, ps, rb, _cs=cs):
                        t = bld.rt_shared[:, mc, :]
                        nc.scalar.activation(t, ps[:, :], AF.Relu)
                        if acc_started[0]:
                            t2 = bld.lt_shared[:, mc, :]
                            nc.vector.scalar_tensor_tensor(
                                t2, t, float(_cs), rb[:, :],
                                op0=ALU.mult, op1=ALU.mult)
                            nc.vector.scalar_tensor_tensor(
                                acc[:, mc, :], t2, 1.0, acc[:, mc, :],
                                op0=ALU.mult, op1=ALU.add)
                        else:
                            nc.vector.scalar_tensor_tensor(
                                acc[:, mc, :], t, float(_cs), rb[:, :],
                                op0=ALU.mult, op1=ALU.mult)
                    bld.mm_site_ln(lnv, w, epi)
                else:
                    def epi(mc, ps, _cs=cs, _bt=bias_t):
                        if acc_started[0]:
                            t = bld.rt_shared[:, mc, :]
                            if _bt is None:
                                bld.relu_ps(t, ps[:, :], _cs)
                            else:
                                nc.scalar.activation(
                                    t, ps[:, :], AF.Relu, scale=float(_cs),
                                    bias=_bt[:, mc:mc + 1])
                            nc.vector.scalar_tensor_tensor(
                                acc[:, mc, :], t, 1.0, acc[:, mc, :],
                                op0=ALU.mult, op1=ALU.add)
                        elif _bt is None:
                            bld.relu_ps(acc[:, mc, :], ps[:, :], _cs)
                        else:
                            nc.scalar.activation(
                                acc[:, mc, :], ps[:, :], AF.Relu,
                                scale=float(_cs), bias=_bt[:, mc:mc + 1])
                    bld.mm_site([(x, w)], epi)
                if not acc_started[0]:
                    acc_mark_started()
                edge_h[e] = ('in_acc', None)
            else:
                cs = s
                out = bld.sb([128, NFC, TOK], BF16, kind="eh")
                if fused:
                    def epi(mc, ps, rb, _c=cs):
                        t = bld.rt_shared[:, mc, :]
                        nc.scalar.activation(t, ps[:, :], AF.Relu)
                        nc.vector.scalar_tensor_tensor(
                            out[:, mc, :], t, float(_c), rb[:, :],
                            op0=ALU.mult, op1=ALU.mult)
                    bld.mm_site_ln(lnv, w, epi)
                elif has_b:
                    bias_t = bld.upload_bias(np.asarray(b) * cs)
                    bld.mm_site([(x, w)],
                                bld.act_epilogue(AF.Relu, out, scale=cs,
                                                 bias_t=bias_t))
                else:
                    bld.mm_site([(x, w)], lambda mc, ps, _c=cs:
                                bld.relu_ps(out[:, mc, :], ps[:, :], _c))
                edge_h[e] = ('relu_scaled', Val(out, 1.0))
        elif ops <= {2, 3} and len(forms) == 1 and direct_to_acc(c, which) \
                and not has_b:
            # linear, single use, straight into the final sum from PSUM
            cs = acc_scale(c, which)
            if fused:
                def epi(mc, ps, rb, _cs=cs):
                    if acc_started[0]:
                        t = bld.rt_shared[:, mc, :]
                        nc.vector.scalar_tensor_tensor(
                            t, ps[:, :], float(_cs), rb[:, :],
                            op0=ALU.mult, op1=ALU.mult)
                        nc.vector.scalar_tensor_tensor(
                            acc[:, mc, :], t, 1.0, acc[:, mc, :],
                            op0=ALU.mult, op1=ALU.add)
                    else:
                        nc.vector.scalar_tensor_tensor(
                            acc[:, mc, :], ps[:, :], float(_cs), rb[:, :],
                            op0=ALU.mult, op1=ALU.mult)
                bld.mm_site_ln(lnv, w, epi)
            else:
                def epi(mc, ps, _cs=cs):
                    acc_add_ps(mc, ps, _cs)
                bld.mm_site([(x, w)], epi)
            if not acc_started[0]:
                acc_mark_started()
            edge_h[e] = ('in_acc', None)
        else:
            # general: materialize h, then any relu/gelu forms
            out = bld.sb([128, NFC, TOK], BF16, kind="eh")
            if fused:
                bld.mm_site_ln(lnv, w, lambda mc, ps, rb:
                               nc.vector.scalar_tensor_tensor(
                                   out[:, mc, :], ps[:, :], 1.0, rb[:, :],
                                   op0=ALU.mult, op1=ALU.mult))
            elif has_b:
                bias_t = bld.upload_bias(b)
                bld.mm_site([(x, w)],
                            bld.act_epilogue(AF.Identity, out,
                                             bias_t=bias_t))
            else:
                bld.mm_site([(x, w)], lambda mc, ps:
                            bld.copy_ps(out[:, mc, :], ps[:, :]))
            edge_h[e] = ('h', Val(out, 1.0))

    def edge_value(c, which):
        """Val for an emitted edge use (h-form resolved per op), with the
        selection scalar NOT yet applied (returned separately)."""
        u = uses[(c, which)]
        if u['op'] == 4:
            v = outs[u['src']]
            return Val(v.t, v.mult * u['s'], v.unit)
        kind, hv = edge_h[u['e']]
        if kind == 'in_acc':
            return None  # already folded into acc
        if kind == 'relu_scaled':
            return Val(hv.t, 1.0)  # scalar already folded
        # kind == 'h'
        if u['op'] in (2, 3):
            return Val(hv.t, u['s'])
        # relu/gelu on materialized h (shared-form edges); unscaled, the
        # selection scalar is returned in the Val
        fkey = (u['e'], u['op'])
        if fkey not in edge_h:
            out = bld.sb([128, NFC, TOK], BF16, kind="ef")
            func = AF.Relu if u['op'] == 0 else AF.Gelu_apprx_tanh
            for fc in range(NFC):
                nc.scalar.activation(out[:, fc, :], hv.t[:, fc, :], func)
            edge_h[fkey] = ('f', Val(out, 1.0))
        fv = edge_h[fkey][1]
        return Val(fv.t, u['s'])

    def prefetch():
        """Emit every not-yet-emitted edge whose input tensor is ready,
        in consumer-node order."""
        for c2 in range(NNOD):
            for which in ('q', 'k', 'v'):
                if (c2, which) not in uses:
                    continue
                u = uses[(c2, which)]
                if u['op'] == 4 or u['e'] in edge_emitted:
                    continue
                if edge_input_ready(u) is not None:
                    emit_edge(c2, which, u)

    # ---- inputs ------------------------------------------------------------
    for nm, idx in (('inpute', -2), ('inputo', -1)):
        if idx in used_src:
            hdl = bld.upload(
                nm,
                [np.ascontiguousarray(
                    np.asarray(np_in[nm]).reshape(-1, ISIZE)
                    [i * TOK:(i + 1) * TOK].astype(ml_dtypes.bfloat16))
                 for i in range(NCORE)],
                [TOK, ISIZE], BF16)
            outs[idx] = bld.load_input_fm(hdl)
            if idx in needs_ln:
                bld.ln_stats(outs[idx])

    # ---- node loop ---------------------------------------------------------
    for c, r in enumerate(routes):
        act = r['act']
        a = aw[c]
        in_rem = c in rem_nodes

        # make sure this node's own edges exist (normally via prefetch)
        for which in ('q', 'k', 'v'):
            if (c, which) in uses and uses[(c, which)]['op'] != 4 \
                    and uses[(c, which)]['e'] not in edge_emitted:
                emit_edge(c, which, uses[(c, which)])

        if act == 7:
            qv = edge_value(c, 'q')
            g, bta = ng[c], nbe[c]
            plain_aff = np.all(g == 1.0) and not np.any(bta)
            needs_tensor = (c in rem_nodes) or any(
                u2['src'] == c and u2['op'] in (3, 4)
                for u2 in uses.values())
            if plain_aff and not needs_tensor and not qv.unit:
                # LN consumed only by fused-LN edges: stats suffice
                raw_of[c] = Val(qv.t, qv.mult, False)
                bld.ln_stats(raw_of[c])
                outs[c] = Val(qv.t, qv.mult, False)
            elif plain_aff:
                ln = bld.ln_of(qv)
                outs[c] = Val(ln.t, ln.mult * a, True)
            else:
                sc = bld.upload_bias(a * ln.mult * g)
                bi = bld.upload_bias(a * bta)
                o = bld.sb([128, NFC, TOK], BF16, kind="n7")
                for fc in range(NFC):
                    nc.scalar.activation(o[:, fc, :], ln.t[:, fc, :],
                                         AF.Identity, scale=sc[:, fc:fc + 1],
                                         bias=bi[:, fc:fc + 1])
                outs[c] = Val(o, 1.0, False)

        elif act == 4:
            # q * sigmoid(k) + v
            u_q, u_k = uses[(c, 'q')], uses[(c, 'k')]
            vv = edge_value(c, 'v')
            shared_g = (u_q['e'] == u_k['e'] and u_q['op'] == 1
                        and u_k['op'] == 1 and vv is not None
                        and edge_h.get(u_q['e'], (None,))[0] == 'h')
            if shared_g:
                # per-chunk pipeline: gelu -> sigmoid -> mul -> combine
                hv = edge_h[u_q['e']][1]
                g = bld.sb([128, NFC, TOK], BF16, kind="g4")
                sg = bld.sb([128, NFC, TOK], BF16, kind="sg")
                m = bld.sb([128, NFC, TOK], BF16, kind="m4")
                o = bld.sb([128, NFC, TOK], BF16, kind="n4")
                edge_h[(u_q['e'], 1)] = ('f', Val(g, 1.0))
                for fc in range(NFC):
                    nc.scalar.activation(g[:, fc, :], hv.t[:, fc, :],
                                         AF.Gelu_apprx_tanh)
                for fc in range(NFC):
                    nc.scalar.activation(sg[:, fc, :], g[:, fc, :],
                                         AF.Sigmoid, scale=float(u_k['s']))
                    nc.vector.tensor_mul(m[:, fc, :], g[:, fc, :],
                                         sg[:, fc, :])
                    nc.vector.scalar_tensor_tensor(
                        o[:, fc, :], m[:, fc, :],
                        float(u_q['s'] / vv.mult), vv.t[:, fc, :],
                        op0=ALU.mult, op1=ALU.add)
                outs[c] = Val(o, a * vv.mult, False)
                if in_rem:
                    acc_add_full(o, a * vv.mult)
            else:
                qv = edge_value(c, 'q')
                kv = edge_value(c, 'k')
                sg = bld.sb([128, NFC, TOK], BF16, kind="sg")
                for fc in range(NFC):
                    nc.scalar.activation(sg[:, fc, :], kv.t[:, fc, :],
                                         AF.Sigmoid, scale=float(kv.mult))
                m = bld.sb([128, NFC, TOK], BF16, kind="m4")
                nc.vector.tensor_mul(m[:, :, :], qv.t[:, :, :], sg[:, :, :])
                if in_rem and vv is None:
                    acc_add_full(m, a * qv.mult)
                    outs[c] = None
                else:
                    o = bld.sb([128, NFC, TOK], BF16, kind="n4")
                    for fc in range(NFC):
                        nc.vector.scalar_tensor_tensor(
                            o[:, fc, :], m[:, fc, :],
                            float(qv.mult / vv.mult), vv.t[:, fc, :],
                            op0=ALU.mult, op1=ALU.add)
                    outs[c] = Val(o, a * vv.mult, False)
                    if in_rem:
                        acc_add_full(o, a * vv.mult)

        elif act == 6:
            # q + k: both either already in acc or added now
            for which in ('q', 'k'):
                u = uses[(c, which)]
                ev = edge_value(c, which)
                if ev is None:
                    continue  # folded into acc from PSUM
                if in_rem:
                    acc_add_full(ev.t, a * ev.mult)
                else:
                    raise NotImplementedError("act6 feeding another node")
            outs[c] = None

        elif act == 5:
            # q + gelu(k@W1 + b1)
            kv = edge_value(c, 'k')
            w1 = nW[c, 1] * kv.mult
            b1 = nb[c, 1]
            bias_t = bld.upload_bias(b1) if np.any(b1) else None
            if in_rem:
                g7 = bld.sb([128, NFC, TOK], BF16, kind="g5")
                def epi(mc, ps):
                    bias_ap = bias_t[:, mc:mc + 1] if bias_t is not None \
                        else 0.0
                    nc.scalar.activation(g7[:, mc, :], ps[:, :],
                                         AF.Gelu_apprx_tanh, bias=bias_ap)
                    nc.vector.scalar_tensor_tensor(
                        acc[:, mc, :], g7[:, mc, :], float(a),
                        acc[:, mc, :], op0=ALU.mult, op1=ALU.add)
                bld.mm_site([(kv.t, w1)], epi)
                qv = edge_value(c, 'q')
                if qv is not None:
                    acc_add_full(qv.t, a * qv.mult)
                outs[c] = None
            else:
                g7 = bld.sb([128, NFC, TOK], BF16, kind="g5")
                bld.mm_site([(kv.t, w1)],
                            bld.act_epilogue(AF.Gelu_apprx_tanh, g7,
                                             bias_t=bias_t))
                qv = edge_value(c, 'q')
                o = bld.sb([128, NFC, TOK], BF16, kind="n5")
                for fc in range(NFC):
                    nc.vector.scalar_tensor_tensor(
                        o[:, fc, :], qv.t[:, fc, :], float(qv.mult),
                        g7[:, fc, :], op0=ALU.mult, op1=ALU.add)
                outs[c] = Val(o, a, False)

        elif act == 3:
            # q + relu(q@W0 + k@W1 + v@W2)@W3 + b3
            qv = edge_value(c, 'q')
            kv = edge_value(c, 'k')
            vv = edge_value(c, 'v')
            inner = bld.sb([128, NFC, TOK], BF16, kind="i3")
            parts = [(qv.t, nW[c, 0] * qv.mult),
                     (kv.t, nW[c, 1] * kv.mult),
                     (vv.t, nW[c, 2] * vv.mult)]
            bld.mm_site(parts, bld.act_epilogue(AF.Relu, inner))
            b3 = nb[c, 3]
            o = bld.sb([128, NFC, TOK], BF16, kind="n3")

            def epi3(mc, ps):
                nc.vector.scalar_tensor_tensor(
                    o[:, mc, :], qv.t[:, mc, :], float(qv.mult), ps[:, :],
                    op0=ALU.mult, op1=ALU.add)
            if np.any(b3):
                bt3 = bld.upload_bias(b3)
                tmp3 = bld.sb([128, NFC, TOK], F32, kind="t3")
                def epi3b(mc, ps):
                    nc.scalar.activation(tmp3[:, mc, :], ps[:, :],
                                         AF.Identity,
                                         bias=bt3[:, mc:mc + 1])
                    nc.vector.scalar_tensor_tensor(
                        o[:, mc, :], qv.t[:, mc, :], float(qv.mult),
                        tmp3[:, mc, :], op0=ALU.mult, op1=ALU.add)
                bld.mm_site([(inner, nW[c, 3])], epi3b)
            else:
                bld.mm_site([(inner, nW[c, 3])], epi3)
            outs[c] = Val(o, a, False)
            if in_rem:
                acc_add_full(o, a)

        elif act == 1:
            # q + (gelu(q@W0+b0) * (k@W1+b1)) @ W3 + b3
            qv = edge_value(c, 'q')
            kv = edge_value(c, 'k')
            g = bld.sb([128, NFC, TOK], BF16, kind="g1")
            b0t = bld.upload_bias(nb[c, 0]) if np.any(nb[c, 0]) else None
            bld.mm_site([(qv.t, nW[c, 0] * qv.mult)],
                        bld.act_epilogue(AF.Gelu_apprx_tanh, g, bias_t=b0t))
            kk = bld.sb([128, NFC, TOK], BF16, kind="k1")
            b1t = bld.upload_bias(nb[c, 1]) if np.any(nb[c, 1]) else None
            bld.mm_site([(kv.t, nW[c, 1] * kv.mult)],
                        bld.act_epilogue(AF.Identity, kk, bias_t=b1t))
            p = bld.sb([128, NFC, TOK], BF16, kind="p1")
            nc.vector.tensor_mul(p[:, :, :], g[:, :, :], kk[:, :, :])
            o = bld.sb([128, NFC, TOK], BF16, kind="n1")
            b3 = nb[c, 3]
            if np.any(b3):
                bt3 = bld.upload_bias(b3)
                tmp1 = bld.sb([128, NFC, TOK], F32, kind="t1")
                def epi1b(mc, ps):
                    nc.scalar.activation(tmp1[:, mc, :], ps[:, :],
                                         AF.Identity, bias=bt3[:, mc:mc + 1])
                    nc.vector.scalar_tensor_tensor(
                        o[:, mc, :], qv.t[:, mc, :], float(qv.mult),
                        tmp1[:, mc, :], op0=ALU.mult, op1=ALU.add)
                bld.mm_site([(p, nW[c, 3])], epi1b)
            else:
                def epi1(mc, ps):
                    nc.vector.scalar_tensor_tensor(
                        o[:, mc, :], qv.t[:, mc, :], float(qv.mult),
                        ps[:, :], op0=ALU.mult, op1=ALU.add)
                bld.mm_site([(p, nW[c, 3])], epi1)
            outs[c] = Val(o, a, False)
            if in_rem:
                acc_add_full(o, a)

        elif act == 2:
            # LN(q + k + v) (+ affine)
            qv = edge_value(c, 'q')
            kv = edge_value(c, 'k')
            vv = edge_value(c, 'v')
            s1 = bld.sb([128, NFC, TOK], BF16, kind="s2a")
            for fc in range(NFC):
                nc.vector.scalar_tensor_tensor(
                    s1[:, fc, :], qv.t[:, fc, :],
                    float(qv.mult / kv.mult), kv.t[:, fc, :],
                    op0=ALU.mult, op1=ALU.add)
            s2t = bld.sb([128, NFC, TOK], BF16, kind="s2b")
            for fc in range(NFC):
                nc.vector.scalar_tensor_tensor(
                    s2t[:, fc, :], vv.t[:, fc, :],
                    float(vv.mult / kv.mult), s1[:, fc, :],
                    op0=ALU.mult, op1=ALU.add)
            sv = Val(s2t, kv.mult, False)
            ln = bld.ln_of(sv)
            outs[c] = Val(ln.t, a, True)
            if in_rem:
                acc_add_full(ln.t, a)

        else:
            raise NotImplementedError(f"act {act}")

        prefetch()
        if c in needs_ln and outs.get(c) is not None and not outs[c].unit:
            bld.ln_of(outs[c])
            prefetch()

    return Val(acc, 1.0, False)


def _emit_final(bld, acc, out_hdl, out_g, out_beta):
    """Transpose to token-major (bf16), per-token LN, DMA out."""
    nc = bld.nc
    xbf = acc.t
    epsp = EPS / (acc.mult * acc.mult)
    need_aff = not (np.all(out_g == 1.0) and not np.any(out_beta))
    if need_aff:
        gh = bld.upload("og", np.tile(np.asarray(out_g, np.float32),
                                      (128, 1)), [128, ISIZE], F32)
        bh = bld.upload("ob", np.tile(np.asarray(out_beta, np.float32),
                                      (128, 1)), [128, ISIZE], F32)
        gt = bld.sb([128, ISIZE], F32, kind="og")
        bt = bld.sb([128, ISIZE], F32, kind="ob")
        nc.sync.dma_start(gt[:, :], gh[:, :])
        nc.sync.dma_start(bt[:, :], bh[:, :])
    eps_col = bld.const_col(epsp, 128)
    fo_tiles = [bld.sb([128, ISIZE], F32, kind="fo") for _ in range(2)]
    for tt in range(NTT):
        ps = bld.ps_bf.tile([128, ISIZE], BF16, tag="psb")
        for fc in range(NFC):
            nc.tensor.transpose(ps[:, ts(fc, 128)], xbf[:, fc, ts(tt, 128)],
                                bld.ident_bf)
        sm = bld.sb([128, 9], F32, kind="fs")
        stats, mv, rstd = sm[:, 0:6], sm[:, 6:8], sm[:, 8:9]
        nc.vector.bn_stats(stats, ps[:, :])
        nc.vector.bn_aggr(mv, stats)
        nc.scalar.activation(rstd, mv[:, 1:2], AF.Ln, bias=eps_col)
        nc.scalar.activation(rstd, rstd, AF.Exp, scale=-0.5)
        ot = fo_tiles[tt % 2]
        nc.vector.tensor_scalar(ot[:, :], ps[:, :], mv[:, 0:1], rstd,
                                op0=ALU.subtract, op1=ALU.mult)
        if need_aff:
            nc.vector.tensor_mul(ot[:, :], ot[:, :], gt[:, :])
            nc.vector.tensor_add(ot[:, :], ot[:, :], bt[:, :])
        nc.sync.dma_start(out_hdl[ts(tt, 128), :], ot[:, :])


def _build_and_run(inputs, trace=False, **run_kwargs):
    np_in = {k: np.asarray(v) for k, v in inputs.items()}
    routes = _routing(np_in['node_p'], np_in['edge_p'])

    nc = bass.Bass(num_devices=NCORE)
    out_hdl = nc.declare_dram_parameter("out", [TOK, ISIZE], F32,
                                        isOutput=True)
    with FixedTileContext(nc) as tc:
        with ExitStack() as ctx:
            bld = Builder(nc, tc, ctx)
            acc = _emit_graph(bld, np_in, routes)
            _emit_final(bld, acc, out_hdl, np.asarray(np_in['out_g']),
                        np.asarray(np_in['out_beta']))
            uploads = bld.uploads
    _hoist_excess_waits(nc)
    in_maps = [{nm: arrs[i] for nm, arrs in uploads.items()}
               for i in range(NCORE)]
    res = run_bass_kernel_spmd(nc, in_maps, core_ids=list(range(NCORE)),
                               trace=trace, **run_kwargs)
    out = np.concatenate([res.results[i]['out'] for i in range(NCORE)], 0)
    return out.reshape(B, SLEN, ISIZE).astype(np.float32), res


def kernel(**inputs):
    out, _ = _build_and_run(inputs)
    return out


# revision 33
# speedup vs baseline: 1.0255x; 1.0255x over previous
"""Trainium2 Bass kernel for nn_DecoderLayer_60060822667509.

Data-parallel over the 4096 tokens (512/core on 8 cores). Routing
(host-side argmax on small logits, mirroring the reference's .item()
syncs) is computed from the actual inputs at call time and a
specialized Bass/Tile program is emitted for the selected DAG.

Design (v2):
- Activations feature-major on-chip ([128 features, NFC chunks, TOK
  tokens]); matmul outputs feed the next matmul's moving operand with
  no transposes.
- LayerNorms are materialized ONCE per source tensor (stats via
  PE ones-matmuls, apply via two DVE passes); every matmul is then a
  plain matmul on a unit-LN tensor with selection/activation scalars
  folded into the bf16 weights host-side.
- All weights are uploaded and DMA'd to SBUF at kernel start in use
  order; nothing is ever spilled to DRAM.
- Edge matmuls are emitted as soon as their source tensor exists
  (lookahead over the route DAG), so the PE queue never head-of-line
  blocks on LN statistics of the node being assembled.
- The final sum (unprocessed nodes) is accumulated in-place in f32 as
  contributions become ready, several directly from PSUM.
"""
import numpy as np
import ml_dtypes
from contextlib import ExitStack

import concourse.bass as bass
import concourse.tile as tile
from concourse import mybir
from concourse.bass import ts
from concourse.bass_utils import run_bass_kernel_spmd
from concourse.masks import make_identity

F32 = mybir.dt.float32
BF16 = mybir.dt.bfloat16
AF = mybir.ActivationFunctionType
ALU = mybir.AluOpType

ISIZE = 512
NNOD = 8
MAXP = 5
TAU = 1.0
EPS = 1e-6
B = 4
SLEN = 1024
NCORE = 8
TOK = (B * SLEN) // NCORE  # 512 tokens per core
NFC = ISIZE // 128         # 4 feature chunks
NTT = TOK // 128           # 4 token tiles


# ---------------------------------------------------------------------------
# Host-side routing (mirrors reference._routing exactly)
# ---------------------------------------------------------------------------

def _qmask(nsrc):
    m = np.zeros((nsrc, 5), bool)
    m[0, :] = True
    return m.reshape(-1)


def _routing(node_p, edge_p):
    node_p = np.asarray(node_p)
    edge_p = np.asarray(edge_p)
    routes, lind = [], 0
    for c in range(NNOD):
        nsrc = min(c + 2, MAXP)
        snode = c - nsrc
        ep = edge_p[:, lind:lind + nsrc, :].reshape(3, -1)
        qm = _qmask(nsrc)
        nact = int(np.argmax(node_p[c]))
        qsel = int(np.argmax(np.where(qm, -np.inf, ep[0])))
        r = dict(lind=lind, nsrc=nsrc, snode=snode, act=nact, q=qsel, k=None,
                 v=None, ktype=None, km=None, vmode=None)
        if nact < 7:
            km = qm if nact > 0 else None
            kl = ep[1] if km is None else np.where(km, -np.inf, ep[1])
            r['k'] = int(np.argmax(kl))
            r['km'] = km
            r['ktype'] = -2 if r['k'] // 5 == 0 else -1
            if nact < 5:
                if nact == 0 and r['ktype'] == -2:
                    r['v'] = int(np.argmax(ep[2][:5]))
                    r['vmode'] = 'first5'
                else:
                    vl = ep[2] if km is None else np.where(km, -np.inf, ep[2])
                    r['v'] = int(np.argmax(vl))
                    r['vmode'] = 'full'
        routes.append(r)
        lind += nsrc
    return routes


def _softmax_np(x):
    x = np.asarray(x, np.float64)
    e = np.exp(x - x.max())
    return e / e.sum()


def _selw_np(logits, mask, sel):
    logits = np.asarray(logits, np.float64)
    if mask is not None:
        logits = np.where(np.asarray(mask), -np.inf, logits)
    return float(_softmax_np(logits / TAU)[sel])


# ---------------------------------------------------------------------------
# TileContext with a walrus-compatible tail drain: this compiler build
# rejects sem waits on SP Drain/NoOp (TPB_CTRL has no wait slots), so
# emit the end-of-kernel waits as standalone wait_ge instructions.
# ---------------------------------------------------------------------------

class FixedTileContext(tile.TileContext):
    def _drain_and_barrier(self, tick_clock, wait_clock):
        nc = self.nc
        clock = list(tick_clock.global_clock)
        for p, sem in sorted(self.sems.allocated().items()):
            c = clock[p]
            if c > 0:
                mult = 16 if sem.name.startswith("DMA") else 1
                nc.sync.wait_ge(sem, c * mult)
        nc.sync.drain()
        nc.all_engine_barrier()
        popped = nc._tile_sem_poison_stack.pop()
        assert popped is self._sem_poison
        nc.clear_and_free_semaphores(list(self.sems.allocated().values()))
        nc.all_engine_barrier()


# ---------------------------------------------------------------------------
# Walrus-compat post-pass: at most one sync wait per engine instruction
# (none on SP control ops). Hoist excess waits onto standalone
# InstEventSemaphore instructions inserted before.
# ---------------------------------------------------------------------------

_NO_HOIST = ("InstEventSemaphore", "InstAllEngineBarrier",
             "InstCollectiveCompute")


def _hoist_excess_waits(nc):
    n = 0
    for f in nc.m.functions:
        for bb in f.blocks:
            out = []
            changed = False
            for inst in bb.instructions:
                tname = type(inst).__name__
                si = inst.sync_info
                if si is not None and tname not in _NO_HOIST:
                    waits = list(si.on_wait)
                    limit = 0 if tname in ("InstDrain", "InstNoOp") else 1
                    if len(waits) > limit:
                        for w in waits[:len(waits) - limit]:
                            n += 1
                            ni = mybir.InstEventSemaphore(
                                name=f"I-hoist{n}", ins=[], outs=[])
                            ni.engine = inst.engine
                            ni.sync_info = mybir.SyncInfo(on_wait=[w],
                                                          on_update=[])
                            out.append(ni)
                        si.on_wait = waits[len(waits) - limit:]
                        changed = True
                out.append(inst)
            if changed:
                bb.instructions = out
    return n


# ---------------------------------------------------------------------------
# Values: SBUF tensor [128, NFC, TOK] plus a symbolic host scalar.
# true value = mult * tensor. unit => tensor is a unit LayerNorm output.
# ---------------------------------------------------------------------------

class Val:
    def __init__(self, t, mult=1.0, unit=False):
        self.t = t
        self.mult = float(mult)
        self.unit = unit


class Builder:
    def __init__(self, nc, tc, ctx):
        self.nc = nc
        self.tc = tc
        self.uploads = {}
        self.n_tag = 0
        self.pool = ctx.enter_context(tc.tile_pool(name="act", bufs=1))
        self.ps_pool = ctx.enter_context(
            tc.tile_pool(name="ps", bufs=4, space="PSUM"))
        self.ps_bf = ctx.enter_context(
            tc.tile_pool(name="psb", bufs=2, space="PSUM"))
        self.ps_stat = ctx.enter_context(
            tc.tile_pool(name="pstat", bufs=2, space="PSUM"))
        self.ident_bf = self.pool.tile([128, 128], BF16, tag="idb")
        ih = self.upload("ident", np.eye(128).astype(ml_dtypes.bfloat16),
                         [128, 128], BF16)
        nc.sync.dma_start(self.ident_bf[:, :], ih[:, :])
        self.ones_bf = self.pool.tile([128, 1], BF16, tag="ones")
        nc.vector.memset(self.ones_bf, 1.0)
        self.ones_row_bf = self.pool.tile([1, 128], BF16, tag="onesr")
        nc.vector.memset(self.ones_row_bf, 1.0)
        self._cc_cache = {}
        self.stats_cache = {}   # id(tensor) -> (rb_sb, mr_sb)
        self.ln_cache = {}      # id(tensor) -> Val (unit LN)
        # shared scratch (serial across stats/LN calls)
        self.sm_shared = self.pool.tile([1, 4 * TOK], F32, tag="smsh")
        self.rm_shared = self.pool.tile([1, 2 * TOK], BF16, tag="rmsh")
        self.x2_shared = self.pool.tile([128, NFC, TOK], BF16, tag="x2sh")
        self.lt_shared = self.pool.tile([128, NFC, TOK], BF16, tag="ltsh")
        self.rt_shared = self.pool.tile([128, NFC, TOK], BF16, tag="rtsh")

    def tag(self, kind="t"):
        self.n_tag += 1
        return f"{kind}{self.n_tag}"

    def sb(self, shape, dtype, kind="a"):
        tg = self.tag(kind)
        return self.pool.tile(list(shape), dtype, tag=tg, name=tg)

    def const_col(self, value, parts=1):
        key = (float(value), parts)
        if key not in self._cc_cache:
            t = self.pool.tile([parts, 1], F32, tag=self.tag("cc"))
            self.nc.vector.memset(t, float(value))
            self._cc_cache[key] = t
        return self._cc_cache[key]

    # -- host->device uploads -----------------------------------------------
    def upload(self, base, arrs, shape, dtype):
        name = f"{base}{len(self.uploads)}"
        if not isinstance(arrs, list):
            arrs = [arrs] * NCORE
        self.uploads[name] = [np.ascontiguousarray(a) for a in arrs]
        return self.nc.declare_dram_parameter(name, list(shape), dtype,
                                              isOutput=False)

    def upload_weight(self, w_np):
        """w_np [512, 512] (in, out) -> bf16 SBUF tile [128, NFC, 512]."""
        arr = np.ascontiguousarray(
            np.asarray(w_np, np.float32).reshape(NFC, 128, ISIZE)
            .transpose(1, 0, 2)).astype(ml_dtypes.bfloat16)
        hdl = self.upload("w", arr, [128, NFC, ISIZE], BF16)
        t = self.sb([128, NFC, ISIZE], BF16, kind="w")
        self.nc.sync.dma_start(t[:, :, :], hdl[:, :, :])
        return t

    def upload_bias(self, b_np):
        """b_np [512] -> SBUF [128, NFC] f32 (per-partition scalars)."""
        arr = np.ascontiguousarray(
            np.asarray(b_np, np.float32).reshape(NFC, 128).transpose(1, 0))
        hdl = self.upload("b", arr, [128, NFC], F32)
        t = self.sb([128, NFC], F32, kind="bias")
        self.nc.sync.dma_start(t[:, :], hdl[:, :])
        return t

    # -- input load ----------------------------------------------------------
    def load_input_fm(self, hdl):
        """DRAM [TOK, 512] bf16 token-major -> feature-major bf16 tensor."""
        nc = self.nc
        out = self.sb([128, NFC, TOK], BF16, kind="in")
        tok_tiles = []
        for tt in range(NTT):
            t = self.sb([128, ISIZE], BF16, kind="int")
            nc.sync.dma_start(t[:, :], hdl[ts(tt, 128), :])
            tok_tiles.append(t)
        for fc in range(NFC):
            ps = self.ps_bf.tile([128, TOK], BF16, tag="psb")
            for tt in range(NTT):
                nc.tensor.transpose(ps[:, ts(tt, 128)],
                                    tok_tiles[tt][:, ts(fc, 128)],
                                    self.ident_bf)
            nc.scalar.activation(out[:, fc, :], ps[:, :], AF.Identity)
        return Val(out, 1.0, False)

    # -- LayerNorm infra -----------------------------------------------------
    def ln_stats(self, val):
        """Per-token stats of the stored tensor: returns (rb_sb, mr_sb),
        both [128, TOK] bf16 broadcasts of rstd' and mean*rstd', such that
        LN(true) = tensor*rb - mr.  eps' = EPS / mult^2."""
        key = id(val.t)
        if key in self.stats_cache:
            return self.stats_cache[key]
        nc = self.nc
        x = val.t
        x2 = self.x2_shared
        m_ps = self.ps_stat.tile([1, TOK], F32, tag="st")
        s2_ps = self.ps_stat.tile([1, TOK], F32, tag="st")
        for kc in range(NFC):
            nc.tensor.matmul(m_ps[:, :], self.ones_bf[:, :], x[:, kc, :],
                             start=(kc == 0), stop=(kc == NFC - 1))
            nc.vector.tensor_mul(x2[:, kc, :], x[:, kc, :], x[:, kc, :])
            nc.tensor.matmul(s2_ps[:, :], self.ones_bf[:, :], x2[:, kc, :],
                             start=(kc == 0), stop=(kc == NFC - 1))
        # mean row (bf16, for the fused mean-correction matmul)
        m_bf = self.sb([1, TOK], BF16, kind="mb")
        nc.vector.tensor_scalar_mul(m_bf[:, :], m_ps[:, :], 1.0 / ISIZE)
        sm = self.sm_shared
        sv = sm[:, 0:TOK]
        nc.vector.scalar_tensor_tensor(sv, m_bf[:, :], -1.0, m_bf[:, :],
                                       op0=ALU.mult, op1=ALU.mult)  # -mean^2
        nc.vector.scalar_tensor_tensor(sv, s2_ps[:, :], 1.0 / ISIZE, sv,
                                       op0=ALU.mult, op1=ALU.add)   # var
        epsp = EPS / (val.mult * val.mult)
        r_bf = self.rm_shared
        nc.scalar.activation(sv, sv, AF.Ln, bias=self.const_col(epsp))
        nc.scalar.activation(r_bf[:, 0:TOK], sv, AF.Exp, scale=-0.5)
        rb_ps = self.ps_pool.tile([128, TOK], F32, tag="ps")
        nc.tensor.matmul(rb_ps[:, :], self.ones_row_bf[:, :],
                         r_bf[:, 0:TOK], start=True, stop=True)
        rb_sb = self.sb([128, TOK], BF16, kind="rb")
        nc.scalar.activation(rb_sb[:, :], rb_ps[:, :], AF.Identity)
        ent = dict(m_bf=m_bf, rb=rb_sb, mr=None)
        self.stats_cache[key] = ent
        return ent

    def ln_mr(self, val):
        """mr broadcast (mean*rstd, [128,TOK] bf16) for materializing."""
        ent = self.ln_stats(val)
        if ent['mr'] is None:
            nc = self.nc
            r_bf = self.rm_shared
            nc.vector.scalar_tensor_tensor(
                r_bf[:, TOK:2 * TOK], ent['m_bf'][:, :], 1.0,
                r_bf[:, 0:TOK], op0=ALU.mult, op1=ALU.mult)
            mr_ps = self.ps_pool.tile([128, TOK], F32, tag="ps")
            nc.tensor.matmul(mr_ps[:, :], self.ones_row_bf[:, :],
                             r_bf[:, TOK:2 * TOK], start=True, stop=True)
            mr_sb = self.sb([128, TOK], BF16, kind="mr")
            nc.scalar.activation(mr_sb[:, :], mr_ps[:, :], AF.Identity)
            ent['mr'] = mr_sb
        return ent

    def ln_of(self, val):
        """Materialized unit-LN of val (cached). Per-chunk two-pass apply:
        u = x*rb - mr."""
        if val.unit:
            kappa = 1.0 / np.sqrt(1.0 + EPS / (val.mult * val.mult))
            return Val(val.t, kappa, True)
        key = id(val.t)
        if key in self.ln_cache:
            return self.ln_cache[key]
        nc = self.nc
        ent = self.ln_mr(val)
        rb_sb, mr_sb = ent['rb'], ent['mr']
        u = self.sb([128, NFC, TOK], BF16, kind="ln")
        tmp = self.lt_shared
        for fc in range(NFC):
            nc.vector.tensor_mul(tmp[:, fc, :], val.t[:, fc, :], rb_sb[:, :])
            nc.vector.scalar_tensor_tensor(
                u[:, fc, :], mr_sb[:, :], -1.0, tmp[:, fc, :],
                op0=ALU.mult, op1=ALU.add)
        out = Val(u, 1.0, True)
        self.ln_cache[key] = out
        return out

    # -- matmul --------------------------------------------------------------
    def mm_site_ln(self, val, w_np, epilogue):
        """Fused-LN matmul: LN(val) @ w, running on the RAW tensor.
        Mean is subtracted inside PSUM via a K=1 matmul with the negated
        column sums of w; rstd is applied in the epilogue, which receives
        (mc, ps, rb)."""
        nc = self.nc
        ent = self.ln_stats(val)
        wbf = np.asarray(w_np, np.float32).astype(ml_dtypes.bfloat16)
        wt = self.upload_weight(wbf)
        wcs = np.ascontiguousarray(
            -wbf.astype(np.float32).sum(axis=0)[None, :]
        ).astype(ml_dtypes.bfloat16)
        hw = self.upload("wc", wcs, [1, ISIZE], BF16)
        wcs_t = self.sb([1, ISIZE], BF16, kind="wc")
        nc.sync.dma_start(wcs_t[:, :], hw[:, :])
        x = val.t
        for mc in range(NFC):
            ps = self.ps_pool.tile([128, TOK], F32, tag="ps")
            for kc in range(NFC):
                nc.tensor.matmul(ps[:, :], wt[:, kc, ts(mc, 128)],
                                 x[:, kc, :], start=(kc == 0), stop=False)
            nc.tensor.matmul(ps[:, :], wcs_t[0:1, ts(mc, 128)],
                             ent['m_bf'][:, :], start=False, stop=True)
            epilogue(mc, ps, ent['rb'])

    def mm_site(self, parts, epilogue):
        """sum_i parts[i] @ W_i accumulated per output chunk; epilogue(mc, ps)
        consumes each chunk's PSUM. parts: list of (tensor, W_np) with all
        scalars folded into W host-side."""
        nc = self.nc
        wts = [self.upload_weight(w) for _, w in parts]
        for mc in range(NFC):
            ps = self.ps_pool.tile([128, TOK], F32, tag="ps")
            n = len(parts) * NFC
            i = 0
            for wt, (x, _) in zip(wts, parts):
                for kc in range(NFC):
                    nc.tensor.matmul(ps[:, :], wt[:, kc, ts(mc, 128)],
                                     x[:, kc, :], start=(i == 0),
                                     stop=(i == n - 1))
                    i += 1
            epilogue(mc, ps)

    def relu_ps(self, out_ap, ps_ap, scale):
        self.nc.scalar.activation(out_ap, ps_ap, AF.Relu,
                                  scale=float(scale))

    def copy_ps(self, out_ap, ps_ap):
        self.nc.scalar.activation(out_ap, ps_ap, AF.Identity)

    def act_epilogue(self, func, out, scale=1.0, bias_t=None):
        """Returns an epilogue writing func(scale*ps + bias) into out."""
        nc = self.nc

        def epi(mc, ps):
            bias_ap = bias_t[:, mc:mc + 1] if bias_t is not None else 0.0
            nc.scalar.activation(out[:, mc, :], ps[:, :], func,
                                 bias=bias_ap, scale=float(scale))
        return epi


# ---------------------------------------------------------------------------
# Graph emission
# ---------------------------------------------------------------------------

def _emit_graph(bld, np_in, routes):
    nc = bld.nc
    eW = np.asarray(np_in['edge_W'], np.float64)
    eb = np.asarray(np_in['edge_b'], np.float64)
    eg = np.asarray(np_in['edge_g'], np.float64)
    ebe = np.asarray(np_in['edge_beta'], np.float64)
    nW = np.asarray(np_in['node_W'], np.float64)
    nb = np.asarray(np_in['node_b'], np.float64)
    ng = np.asarray(np_in['node_g'], np.float64)
    nbe = np.asarray(np_in['node_beta'], np.float64)
    node_p = np.asarray(np_in['node_p'], np.float64)
    edge_p = np.asarray(np_in['edge_p'], np.float64)

    for r in routes:
        assert r['act'] != 0, "attention routing not supported in v2 kernel"

    # ---- route analysis ----------------------------------------------------
    # edge list: one entry per (consumer c, which) with selection scalar.
    # uses[(c, which)] = dict(src, e, op, s)
    uses = {}
    processed = set()
    used_src = set()
    for c, r in enumerate(routes):
        lind, nsrc = r['lind'], r['nsrc']
        ep = edge_p[:, lind:lind + nsrc, :].reshape(3, -1)
        for which, sel in (('q', r['q']), ('k', r['k']), ('v', r['v'])):
            if sel is None:
                continue
            se, op = sel // 5, sel % 5
            src = -2 if se == 0 else r['snode'] + se
            logits = ep[{'q': 0, 'k': 1, 'v': 2}[which]]
            first5 = (which == 'v' and r['vmode'] == 'first5')
            if first5:
                logits = logits[:5]
            mask = _qmask(nsrc) if which == 'q' else r['km']
            if first5:
                mask = None
            s = _selw_np(logits, mask, sel)
            uses[(c, which)] = dict(src=src, e=lind + se, op=op, s=s)
            processed.add(src)
            used_src.add(src)

    # which sources need LN (feed op<=2 edges)
    needs_ln = {u['src'] for u in uses.values() if u['op'] <= 2}
    aw = {c: float(_softmax_np(node_p[c] / TAU)[routes[c]['act']])
          for c in range(NNOD)}

    # final-sum membership: nodes never consumed as a source
    rem_nodes = [i for i in range(NNOD) if i not in processed]

    # ---- value bookkeeping -------------------------------------------------
    outs = {}          # node idx -> Val
    raw_of = {}        # node idx -> pre-LN raw Val (for fused-LN consumers)
    edge_h = {}        # e -> Val  (raw h of LN-edge or linear edge, unscaled)
    edge_emitted = set()

    # acc: the final sum, accumulated in-place, f32, true scale
    acc = bld.sb([128, NFC, TOK], BF16, kind="acc")
    acc_started = [False]

    def acc_add_ps(mc, ps, scale=1.0):
        """acc[:, mc, :] += scale * ps   (or initialize)."""
        if not acc_started[0]:
            nc.scalar.activation(acc[:, mc, :], ps[:, :], AF.Identity,
                                 scale=float(scale))
        else:
            nc.vector.scalar_tensor_tensor(
                acc[:, mc, :], ps[:, :], float(scale), acc[:, mc, :],
                op0=ALU.mult, op1=ALU.add)

    def acc_add_full(x, scale):
        """acc += scale * x (full tile, SBUF tensor)."""
        assert acc_started[0]
        for fc in range(NFC):
            nc.vector.scalar_tensor_tensor(
                acc[:, fc, :], x[:, fc, :], float(scale), acc[:, fc, :],
                op0=ALU.mult, op1=ALU.add)

    def acc_mark_started():
        acc_started[0] = True

    # does this (c, which) use feed the final accumulator directly?
    # -> node c is in rem AND its act combines terms additively for this slot
    def direct_to_acc(c, which):
        if c not in rem_nodes:
            return False
        a = routes[c]['act']
        # act6: q + k ; act5: q + gelu(k@W1+b1) (q slot only)
        # act4: q*sig(k) + v (v slot only)
        return (a == 6) or (a == 5 and which == 'q') or \
               (a == 4 and which == 'v')

    # multiplier applied to node c's term for `which` inside the final sum
    def acc_scale(c, which):
        return aw[c] * uses[(c, which)]['s']

    # ---- edge emission -----------------------------------------------------
    def w_eff(u):
        """Effective weight for an edge use (LN affine folded; for op3 the
        source mult is folded by the caller)."""
        e, op = u['e'], u['op']
        if op <= 2:
            return eg[e][:, None] * eW[e]
        return eW[e]

    def b_eff(u):
        e, op = u['e'], u['op']
        if op <= 2:
            return ebe[e] @ eW[e] + eb[e]
        return eb[e]

    def edge_input_ready(u):
        """(val, fused) the edge's matmul streams, or None if not ready.
        For LN edges on a non-unit source (or one with a recorded raw
        tensor), the matmul fuses the LN on the raw tensor."""
        src = u['src']
        if src not in outs:
            return None
        v = outs[src]
        if u['op'] in (0, 1, 2):
            if src in raw_of:
                return (raw_of[src], True)
            if not v.unit:
                return (v, True)
            return (bld.ln_of(v), False)
        return (v, False)

    def emit_edge(c, which, u):
        """Emit matmul + epilogue for one (consumer, which) use."""
        op, e, s = u['op'], u['e'], u['s']
        src_v = outs[u['src']]
        if op == 4:
            return  # identity: no work
        ready = edge_input_ready(u)
        assert ready is not None
        lnv, fused = ready
        x, xmult = lnv.t, lnv.mult
        if fused:
            xmult = 1.0  # LN of the raw tensor is unit by construction
        # collect all uses sharing this e (same consumer & src by construct)
        forms = [(w2, u2) for (c2, w2), u2 in uses.items()
                 if c2 == c and u2['e'] == e and u2['op'] != 4]
        key = e
        if key in edge_emitted:
            return
        edge_emitted.add(key)
        ops = {u2['op'] for _, u2 in forms}
        w = w_eff(u) * xmult
        b = b_eff(u)
        has_b = bool(np.any(b))

        if fused and has_b:
            fused = False
            lnv2 = bld.ln_of(lnv)
            x, xmult = lnv2.t, lnv2.mult
            w = w_eff(u) * xmult

        if ops == {0} and len(forms) == 1:
            # relu-only: fold consumer scalar (>0) through relu; the bias is
            # pre-scaled so Relu(cs*ps + cs*b) = cs*relu(ps + b)
            if direct_to_acc(c, which):
                cs = acc_scale(c, which)
                bias_t = bld.upload_bias(np.asarray(b) * cs) if has_b else None
                if fused:
                    # relu(rb*ps)*cs = rb>0 -> cs*rb*relu(ps)
                    def epi(mc, ps, rb, _cs=cs):
                        t = bld.rt_shared[:, mc, :]
                        nc.scalar.activation(t, ps[:, :], AF.Relu)
                        if acc_started[0]:
                            t2 = bld.lt_shared[:, mc, :]
                            nc.vector.scalar_tensor_tensor(
                                t2, t, float(_cs), rb[:, :],
                                op0=ALU.mult, op1=ALU.mult)
                            nc.vector.scalar_tensor_tensor(
                                acc[:, mc, :], t2, 1.0, acc[:, mc, :],
                                op0=ALU.mult, op1=ALU.add)
                        else:
                            nc.vector.scalar_tensor_tensor(
                                acc[:, mc, :], t, float(_cs), rb[:, :],
                                op0=ALU.mult, op1=ALU.mult)
                    bld.mm_site_ln(lnv, w, epi)
                else:
                    def epi(mc, ps, _cs=cs, _bt=bias_t):
                        if acc_started[0]:
                            t = bld.rt_shared[:, mc, :]
                            if _bt is None:
                                bld.relu_ps(t, ps[:, :], _cs)
                            else:
                                nc.scalar.activation(
                                    t, ps[:, :], AF.Relu, scale=float(_cs),
                                    bias=_bt[:, mc:mc + 1])
                            nc.vector.scalar_tensor_tensor(
                                acc[:, mc, :], t, 1.0, acc[:, mc, :],
                                op0=ALU.mult, op1=ALU.add)
                        elif _bt is None:
                            bld.relu_ps(acc[:, mc, :], ps[:, :], _cs)
                        else:
                            nc.scalar.activation(
                                acc[:, mc, :], ps[:, :], AF.Relu,
                                scale=float(_cs), bias=_bt[:, mc:mc + 1])
                    bld.mm_site([(x, w)], epi)
                if not acc_started[0]:
                    acc_mark_started()
                edge_h[e] = ('in_acc', None)
            else:
                cs = s
                out = bld.sb([128, NFC, TOK], BF16, kind="eh")
                if fused:
                    def epi(mc, ps, rb, _c=cs):
                        t = bld.rt_shared[:, mc, :]
                        nc.scalar.activation(t, ps[:, :], AF.Relu)
                        nc.vector.scalar_tensor_tensor(
                            out[:, mc, :], t, float(_c), rb[:, :],
                            op0=ALU.mult, op1=ALU.mult)
                    bld.mm_site_ln(lnv, w, epi)
                elif has_b:
                    bias_t = bld.upload_bias(np.asarray(b) * cs)
                    bld.mm_site([(x, w)],
                                bld.act_epilogue(AF.Relu, out, scale=cs,
                                                 bias_t=bias_t))
                else:
                    bld.mm_site([(x, w)], lambda mc, ps, _c=cs:
                                bld.relu_ps(out[:, mc, :], ps[:, :], _c))
                edge_h[e] = ('relu_scaled', Val(out, 1.0))
        elif ops <= {2, 3} and len(forms) == 1 and direct_to_acc(c, which) \
                and not has_b:
            # linear, single use, straight into the final sum from PSUM
            cs = acc_scale(c, which)
            if fused:
                def epi(mc, ps, rb, _cs=cs):
                    if acc_started[0]:
                        t = bld.rt_shared[:, mc, :]
                        nc.vector.scalar_tensor_tensor(
                            t, ps[:, :], float(_cs), rb[:, :],
                            op0=ALU.mult, op1=ALU.mult)
                        nc.vector.scalar_tensor_tensor(
                            acc[:, mc, :], t, 1.0, acc[:, mc, :],
                            op0=ALU.mult, op1=ALU.add)
                    else:
                        nc.vector.scalar_tensor_tensor(
                            acc[:, mc, :], ps[:, :], float(_cs), rb[:, :],
                            op0=ALU.mult, op1=ALU.mult)
                bld.mm_site_ln(lnv, w, epi)
            else:
                def epi(mc, ps, _cs=cs):
                    acc_add_ps(mc, ps, _cs)
                bld.mm_site([(x, w)], epi)
            if not acc_started[0]:
                acc_mark_started()
            edge_h[e] = ('in_acc', None)
        else:
            # general: materialize h, then any relu/gelu forms
            out = bld.sb([128, NFC, TOK], BF16, kind="eh")
            if fused:
                bld.mm_site_ln(lnv, w, lambda mc, ps, rb:
                               nc.vector.scalar_tensor_tensor(
                                   out[:, mc, :], ps[:, :], 1.0, rb[:, :],
                                   op0=ALU.mult, op1=ALU.mult))
            elif has_b:
                bias_t = bld.upload_bias(b)
                bld.mm_site([(x, w)],
                            bld.act_epilogue(AF.Identity, out,
                                             bias_t=bias_t))
            else:
                bld.mm_site([(x, w)], lambda mc, ps:
                            bld.copy_ps(out[:, mc, :], ps[:, :]))
            edge_h[e] = ('h', Val(out, 1.0))

    def edge_value(c, which):
        """Val for an emitted edge use (h-form resolved per op), with the
        selection scalar NOT yet applied (returned separately)."""
        u = uses[(c, which)]
        if u['op'] == 4:
            v = outs[u['src']]
            return Val(v.t, v.mult * u['s'], v.unit)
        kind, hv = edge_h[u['e']]
        if kind == 'in_acc':
            return None  # already folded into acc
        if kind == 'relu_scaled':
            return Val(hv.t, 1.0)  # scalar already folded
        # kind == 'h'
        if u['op'] in (2, 3):
            return Val(hv.t, u['s'])
        # relu/gelu on materialized h (shared-form edges); unscaled, the
        # selection scalar is returned in the Val
        fkey = (u['e'], u['op'])
        if fkey not in edge_h:
            out = bld.sb([128, NFC, TOK], BF16, kind="ef")
            func = AF.Relu if u['op'] == 0 else AF.Gelu_apprx_tanh
            for fc in range(NFC):
                nc.scalar.activation(out[:, fc, :], hv.t[:, fc, :], func)
            edge_h[fkey] = ('f', Val(out, 1.0))
        fv = edge_h[fkey][1]
        return Val(fv.t, u['s'])

    def prefetch():
        """Emit every not-yet-emitted edge whose input tensor is ready,
        in consumer-node order."""
        for c2 in range(NNOD):
            for which in ('q', 'k', 'v'):
                if (c2, which) not in uses:
                    continue
                u = uses[(c2, which)]
                if u['op'] == 4 or u['e'] in edge_emitted:
                    continue
                if edge_input_ready(u) is not None:
                    emit_edge(c2, which, u)

    # ---- inputs ------------------------------------------------------------
    for nm, idx in (('inpute', -2), ('inputo', -1)):
        if idx in used_src:
            hdl = bld.upload(
                nm,
                [np.ascontiguousarray(
                    np.asarray(np_in[nm]).reshape(-1, ISIZE)
                    [i * TOK:(i + 1) * TOK].astype(ml_dtypes.bfloat16))
                 for i in range(NCORE)],
                [TOK, ISIZE], BF16)
            outs[idx] = bld.load_input_fm(hdl)
            if idx in needs_ln:
                bld.ln_stats(outs[idx])

    # ---- node loop ---------------------------------------------------------
    for c, r in enumerate(routes):
        act = r['act']
        a = aw[c]
        in_rem = c in rem_nodes

        # make sure this node's own edges exist (normally via prefetch)
        for which in ('q', 'k', 'v'):
            if (c, which) in uses and uses[(c, which)]['op'] != 4 \
                    and uses[(c, which)]['e'] not in edge_emitted:
                emit_edge(c, which, uses[(c, which)])

        if act == 7:
            qv = edge_value(c, 'q')
            g, bta = ng[c], nbe[c]
            plain_aff = np.all(g == 1.0) and not np.any(bta)
            needs_tensor = (c in rem_nodes) or any(
                u2['src'] == c and u2['op'] in (3, 4)
                for u2 in uses.values())
            if plain_aff and not needs_tensor and not qv.unit:
                # LN consumed only by fused-LN edges: stats suffice
                raw_of[c] = Val(qv.t, qv.mult, False)
                bld.ln_stats(raw_of[c])
                outs[c] = Val(qv.t, qv.mult, False)
            elif plain_aff:
                ln = bld.ln_of(qv)
                if not qv.unit:
                    raw_of[c] = qv
                outs[c] = Val(ln.t, ln.mult * a, True)
            else:
                sc = bld.upload_bias(a * ln.mult * g)
                bi = bld.upload_bias(a * bta)
                o = bld.sb([128, NFC, TOK], BF16, kind="n7")
                for fc in range(NFC):
                    nc.scalar.activation(o[:, fc, :], ln.t[:, fc, :],
                                         AF.Identity, scale=sc[:, fc:fc + 1],
                                         bias=bi[:, fc:fc + 1])
                outs[c] = Val(o, 1.0, False)

        elif act == 4:
            # q * sigmoid(k) + v
            u_q, u_k = uses[(c, 'q')], uses[(c, 'k')]
            vv = edge_value(c, 'v')
            shared_g = (u_q['e'] == u_k['e'] and u_q['op'] == 1
                        and u_k['op'] == 1 and vv is not None
                        and edge_h.get(u_q['e'], (None,))[0] == 'h')
            if shared_g:
                # per-chunk pipeline: gelu -> sigmoid -> mul -> combine
                hv = edge_h[u_q['e']][1]
                g = bld.sb([128, NFC, TOK], BF16, kind="g4")
                sg = bld.sb([128, NFC, TOK], BF16, kind="sg")
                m = bld.sb([128, NFC, TOK], BF16, kind="m4")
                o = bld.sb([128, NFC, TOK], BF16, kind="n4")
                edge_h[(u_q['e'], 1)] = ('f', Val(g, 1.0))
                for fc in range(NFC):
                    nc.scalar.activation(g[:, fc, :], hv.t[:, fc, :],
                                         AF.Gelu_apprx_tanh)
                for fc in range(NFC):
                    nc.scalar.activation(sg[:, fc, :], g[:, fc, :],
                                         AF.Sigmoid, scale=float(u_k['s']))
                    nc.vector.tensor_mul(m[:, fc, :], g[:, fc, :],
                                         sg[:, fc, :])
                    nc.vector.scalar_tensor_tensor(
                        o[:, fc, :], m[:, fc, :],
                        float(u_q['s'] / vv.mult), vv.t[:, fc, :],
                        op0=ALU.mult, op1=ALU.add)
                outs[c] = Val(o, a * vv.mult, False)
                if in_rem:
                    acc_add_full(o, a * vv.mult)
            else:
                qv = edge_value(c, 'q')
                kv = edge_value(c, 'k')
                sg = bld.sb([128, NFC, TOK], BF16, kind="sg")
                for fc in range(NFC):
                    nc.scalar.activation(sg[:, fc, :], kv.t[:, fc, :],
                                         AF.Sigmoid, scale=float(kv.mult))
                m = bld.sb([128, NFC, TOK], BF16, kind="m4")
                nc.vector.tensor_mul(m[:, :, :], qv.t[:, :, :], sg[:, :, :])
                if in_rem and vv is None:
                    acc_add_full(m, a * qv.mult)
                    outs[c] = None
                else:
                    o = bld.sb([128, NFC, TOK], BF16, kind="n4")
                    for fc in range(NFC):
                        nc.vector.scalar_tensor_tensor(
                            o[:, fc, :], m[:, fc, :],
                            float(qv.mult / vv.mult), vv.t[:, fc, :],
                            op0=ALU.mult, op1=ALU.add)
                    outs[c] = Val(o, a * vv.mult, False)
                    if in_rem:
                        acc_add_full(o, a * vv.mult)

        elif act == 6:
            # q + k: both either already in acc or added now
            for which in ('q', 'k'):
                u = uses[(c, which)]
                ev = edge_value(c, which)
                if ev is None:
                    continue  # folded into acc from PSUM
                if in_rem:
                    acc_add_full(ev.t, a * ev.mult)
                else:
                    raise NotImplementedError("act6 feeding another node")
            outs[c] = None

        elif act == 5:
            # q + gelu(k@W1 + b1)
            kv = edge_value(c, 'k')
            w1 = nW[c, 1] * kv.mult
            b1 = nb[c, 1]
            bias_t = bld.upload_bias(b1) if np.any(b1) else None
            if in_rem:
                g7 = bld.sb([128, NFC, TOK], BF16, kind="g5")
                def epi(mc, ps):
                    bias_ap = bias_t[:, mc:mc + 1] if bias_t is not None \
                        else 0.0
                    nc.scalar.activation(g7[:, mc, :], ps[:, :],
                                         AF.Gelu_apprx_tanh, bias=bias_ap)
                    nc.vector.scalar_tensor_tensor(
                        acc[:, mc, :], g7[:, mc, :], float(a),
                        acc[:, mc, :], op0=ALU.mult, op1=ALU.add)
                bld.mm_site([(kv.t, w1)], epi)
                qv = edge_value(c, 'q')
                if qv is not None:
                    acc_add_full(qv.t, a * qv.mult)
                outs[c] = None
            else:
                g7 = bld.sb([128, NFC, TOK], BF16, kind="g5")
                bld.mm_site([(kv.t, w1)],
                            bld.act_epilogue(AF.Gelu_apprx_tanh, g7,
                                             bias_t=bias_t))
                qv = edge_value(c, 'q')
                o = bld.sb([128, NFC, TOK], BF16, kind="n5")
                for fc in range(NFC):
                    nc.vector.scalar_tensor_tensor(
                        o[:, fc, :], qv.t[:, fc, :], float(qv.mult),
                        g7[:, fc, :], op0=ALU.mult, op1=ALU.add)
                outs[c] = Val(o, a, False)

        elif act == 3:
            # q + relu(q@W0 + k@W1 + v@W2)@W3 + b3
            qv = edge_value(c, 'q')
            kv = edge_value(c, 'k')
            vv = edge_value(c, 'v')
            inner = bld.sb([128, NFC, TOK], BF16, kind="i3")
            parts = [(qv.t, nW[c, 0] * qv.mult),
                     (kv.t, nW[c, 1] * kv.mult),
                     (vv.t, nW[c, 2] * vv.mult)]
            bld.mm_site(parts, bld.act_epilogue(AF.Relu, inner))
            b3 = nb[c, 3]
            o = bld.sb([128, NFC, TOK], BF16, kind="n3")

            def epi3(mc, ps):
                nc.vector.scalar_tensor_tensor(
                    o[:, mc, :], qv.t[:, mc, :], float(qv.mult), ps[:, :],
                    op0=ALU.mult, op1=ALU.add)
            if np.any(b3):
                bt3 = bld.upload_bias(b3)
                tmp3 = bld.sb([128, NFC, TOK], F32, kind="t3")
                def epi3b(mc, ps):
                    nc.scalar.activation(tmp3[:, mc, :], ps[:, :],
                                         AF.Identity,
                                         bias=bt3[:, mc:mc + 1])
                    nc.vector.scalar_tensor_tensor(
                        o[:, mc, :], qv.t[:, mc, :], float(qv.mult),
                        tmp3[:, mc, :], op0=ALU.mult, op1=ALU.add)
                bld.mm_site([(inner, nW[c, 3])], epi3b)
            else:
                bld.mm_site([(inner, nW[c, 3])], epi3)
            outs[c] = Val(o, a, False)
            if in_rem:
                acc_add_full(o, a)

        elif act == 1:
            # q + (gelu(q@W0+b0) * (k@W1+b1)) @ W3 + b3
            qv = edge_value(c, 'q')
            kv = edge_value(c, 'k')
            g = bld.sb([128, NFC, TOK], BF16, kind="g1")
            b0t = bld.upload_bias(nb[c, 0]) if np.any(nb[c, 0]) else None
            bld.mm_site([(qv.t, nW[c, 0] * qv.mult)],
                        bld.act_epilogue(AF.Gelu_apprx_tanh, g, bias_t=b0t))
            kk = bld.sb([128, NFC, TOK], BF16, kind="k1")
            b1t = bld.upload_bias(nb[c, 1]) if np.any(nb[c, 1]) else None
            bld.mm_site([(kv.t, nW[c, 1] * kv.mult)],
                        bld.act_epilogue(AF.Identity, kk, bias_t=b1t))
            p = bld.sb([128, NFC, TOK], BF16, kind="p1")
            nc.vector.tensor_mul(p[:, :, :], g[:, :, :], kk[:, :, :])
            o = bld.sb([128, NFC, TOK], BF16, kind="n1")
            b3 = nb[c, 3]
            if np.any(b3):
                bt3 = bld.upload_bias(b3)
                tmp1 = bld.sb([128, NFC, TOK], F32, kind="t1")
                def epi1b(mc, ps):
                    nc.scalar.activation(tmp1[:, mc, :], ps[:, :],
                                         AF.Identity, bias=bt3[:, mc:mc + 1])
                    nc.vector.scalar_tensor_tensor(
                        o[:, mc, :], qv.t[:, mc, :], float(qv.mult),
                        tmp1[:, mc, :], op0=ALU.mult, op1=ALU.add)
                bld.mm_site([(p, nW[c, 3])], epi1b)
            else:
                def epi1(mc, ps):
                    nc.vector.scalar_tensor_tensor(
                        o[:, mc, :], qv.t[:, mc, :], float(qv.mult),
                        ps[:, :], op0=ALU.mult, op1=ALU.add)
                bld.mm_site([(p, nW[c, 3])], epi1)
            outs[c] = Val(o, a, False)
            if in_rem:
                acc_add_full(o, a)

        elif act == 2:
            # LN(q + k + v) (+ affine)
            qv = edge_value(c, 'q')
            kv = edge_value(c, 'k')
            vv = edge_value(c, 'v')
            s1 = bld.sb([128, NFC, TOK], BF16, kind="s2a")
            for fc in range(NFC):
                nc.vector.scalar_tensor_tensor(
                    s1[:, fc, :], qv.t[:, fc, :],
                    float(qv.mult / kv.mult), kv.t[:, fc, :],
                    op0=ALU.mult, op1=ALU.add)
            s2t = bld.sb([128, NFC, TOK], BF16, kind="s2b")
            for fc in range(NFC):
                nc.vector.scalar_tensor_tensor(
                    s2t[:, fc, :], vv.t[:, fc, :],
                    float(vv.mult / kv.mult), s1[:, fc, :],
                    op0=ALU.mult, op1=ALU.add)
            sv = Val(s2t, kv.mult, False)
            ln = bld.ln_of(sv)
            outs[c] = Val(ln.t, a, True)
            if in_rem:
                acc_add_full(ln.t, a)

        else:
            raise NotImplementedError(f"act {act}")

        prefetch()
        if c in needs_ln and outs.get(c) is not None and not outs[c].unit:
            bld.ln_of(outs[c])
            prefetch()

    return Val(acc, 1.0, False)


def _emit_final(bld, acc, out_hdl, out_g, out_beta):
    """Transpose to token-major (bf16), per-token LN, DMA out."""
    nc = bld.nc
    xbf = acc.t
    epsp = EPS / (acc.mult * acc.mult)
    need_aff = not (np.all(out_g == 1.0) and not np.any(out_beta))
    if need_aff:
        gh = bld.upload("og", np.tile(np.asarray(out_g, np.float32),
                                      (128, 1)), [128, ISIZE], F32)
        bh = bld.upload("ob", np.tile(np.asarray(out_beta, np.float32),
                                      (128, 1)), [128, ISIZE], F32)
        gt = bld.sb([128, ISIZE], F32, kind="og")
        bt = bld.sb([128, ISIZE], F32, kind="ob")
        nc.sync.dma_start(gt[:, :], gh[:, :])
        nc.sync.dma_start(bt[:, :], bh[:, :])
    eps_col = bld.const_col(epsp, 128)
    fo_tiles = [bld.sb([128, ISIZE], F32, kind="fo") for _ in range(2)]
    for tt in range(NTT):
        ps = bld.ps_bf.tile([128, ISIZE], BF16, tag="psb")
        for fc in range(NFC):
            nc.tensor.transpose(ps[:, ts(fc, 128)], xbf[:, fc, ts(tt, 128)],
                                bld.ident_bf)
        sm = bld.sb([128, 9], F32, kind="fs")
        stats, mv, rstd = sm[:, 0:6], sm[:, 6:8], sm[:, 8:9]
        nc.vector.bn_stats(stats, ps[:, :])
        nc.vector.bn_aggr(mv, stats)
        nc.scalar.activation(rstd, mv[:, 1:2], AF.Ln, bias=eps_col)
        nc.scalar.activation(rstd, rstd, AF.Exp, scale=-0.5)
        ot = fo_tiles[tt % 2]
        nc.vector.tensor_scalar(ot[:, :], ps[:, :], mv[:, 0:1], rstd,
                                op0=ALU.subtract, op1=ALU.mult)
        if need_aff:
            nc.vector.tensor_mul(ot[:, :], ot[:, :], gt[:, :])
            nc.vector.tensor_add(ot[:, :], ot[:, :], bt[:, :])
        nc.sync.dma_start(out_hdl[ts(tt, 128), :], ot[:, :])


def _build_and_run(inputs, trace=False, **run_kwargs):
    np_in = {k: np.asarray(v) for k, v in inputs.items()}
    routes = _routing(np_in['node_p'], np_in['edge_p'])

    nc = bass.Bass(num_devices=NCORE)
    out_hdl = nc.declare_dram_parameter("out", [TOK, ISIZE], F32,
                                        isOutput=True)
    with FixedTileContext(nc) as tc:
        with ExitStack() as ctx:
            bld = Builder(nc, tc, ctx)
            acc = _emit_graph(bld, np_in, routes)
            _emit_final(bld, acc, out_hdl, np.asarray(np_in['out_g']),
                        np.asarray(np_in['out_beta']))
            uploads = bld.uploads
    _hoist_excess_waits(nc)
    in_maps = [{nm: arrs[i] for nm, arrs in uploads.items()}
               for i in range(NCORE)]
    res = run_bass_kernel_spmd(nc, in_maps, core_ids=list(range(NCORE)),
                               trace=trace, **run_kwargs)
    out = np.concatenate([res.results[i]['out'] for i in range(NCORE)], 0)
    return out.reshape(B, SLEN, ISIZE).astype(np.float32), res


def kernel(**inputs):
    out, _ = _build_and_run(inputs)
    return out


# revision 35
# speedup vs baseline: 1.0641x; 1.0377x over previous
"""Trainium2 Bass kernel for nn_DecoderLayer_60060822667509.

Data-parallel over the 4096 tokens (512/core on 8 cores). Routing
(host-side argmax on small logits, mirroring the reference's .item()
syncs) is computed from the actual inputs at call time and a
specialized Bass/Tile program is emitted for the selected DAG.

Design (v2):
- Activations feature-major on-chip ([128 features, NFC chunks, TOK
  tokens]); matmul outputs feed the next matmul's moving operand with
  no transposes.
- LayerNorms are materialized ONCE per source tensor (stats via
  PE ones-matmuls, apply via two DVE passes); every matmul is then a
  plain matmul on a unit-LN tensor with selection/activation scalars
  folded into the bf16 weights host-side.
- All weights are uploaded and DMA'd to SBUF at kernel start in use
  order; nothing is ever spilled to DRAM.
- Edge matmuls are emitted as soon as their source tensor exists
  (lookahead over the route DAG), so the PE queue never head-of-line
  blocks on LN statistics of the node being assembled.
- The final sum (unprocessed nodes) is accumulated in-place in f32 as
  contributions become ready, several directly from PSUM.
"""
import numpy as np
import ml_dtypes
from contextlib import ExitStack

import concourse.bass as bass
import concourse.tile as tile
from concourse import mybir
from concourse.bass import ts
from concourse.bass_utils import run_bass_kernel_spmd
from concourse.masks import make_identity

F32 = mybir.dt.float32
BF16 = mybir.dt.bfloat16
AF = mybir.ActivationFunctionType
ALU = mybir.AluOpType

ISIZE = 512
NNOD = 8
MAXP = 5
TAU = 1.0
EPS = 1e-6
B = 4
SLEN = 1024
NCORE = 8
TOK = (B * SLEN) // NCORE  # 512 tokens per core
NFC = ISIZE // 128         # 4 feature chunks
NTT = TOK // 128           # 4 token tiles


# ---------------------------------------------------------------------------
# Host-side routing (mirrors reference._routing exactly)
# ---------------------------------------------------------------------------

def _qmask(nsrc):
    m = np.zeros((nsrc, 5), bool)
    m[0, :] = True
    return m.reshape(-1)


def _routing(node_p, edge_p):
    node_p = np.asarray(node_p)
    edge_p = np.asarray(edge_p)
    routes, lind = [], 0
    for c in range(NNOD):
        nsrc = min(c + 2, MAXP)
        snode = c - nsrc
        ep = edge_p[:, lind:lind + nsrc, :].reshape(3, -1)
        qm = _qmask(nsrc)
        nact = int(np.argmax(node_p[c]))
        qsel = int(np.argmax(np.where(qm, -np.inf, ep[0])))
        r = dict(lind=lind, nsrc=nsrc, snode=snode, act=nact, q=qsel, k=None,
                 v=None, ktype=None, km=None, vmode=None)
        if nact < 7:
            km = qm if nact > 0 else None
            kl = ep[1] if km is None else np.where(km, -np.inf, ep[1])
            r['k'] = int(np.argmax(kl))
            r['km'] = km
            r['ktype'] = -2 if r['k'] // 5 == 0 else -1
            if nact < 5:
                if nact == 0 and r['ktype'] == -2:
                    r['v'] = int(np.argmax(ep[2][:5]))
                    r['vmode'] = 'first5'
                else:
                    vl = ep[2] if km is None else np.where(km, -np.inf, ep[2])
                    r['v'] = int(np.argmax(vl))
                    r['vmode'] = 'full'
        routes.append(r)
        lind += nsrc
    return routes


def _softmax_np(x):
    x = np.asarray(x, np.float64)
    e = np.exp(x - x.max())
    return e / e.sum()


def _selw_np(logits, mask, sel):
    logits = np.asarray(logits, np.float64)
    if mask is not None:
        logits = np.where(np.asarray(mask), -np.inf, logits)
    return float(_softmax_np(logits / TAU)[sel])


# ---------------------------------------------------------------------------
# TileContext with a walrus-compatible tail drain: this compiler build
# rejects sem waits on SP Drain/NoOp (TPB_CTRL has no wait slots), so
# emit the end-of-kernel waits as standalone wait_ge instructions.
# ---------------------------------------------------------------------------

class FixedTileContext(tile.TileContext):
    def _drain_and_barrier(self, tick_clock, wait_clock):
        nc = self.nc
        clock = list(tick_clock.global_clock)
        for p, sem in sorted(self.sems.allocated().items()):
            c = clock[p]
            if c > 0:
                mult = 16 if sem.name.startswith("DMA") else 1
                nc.sync.wait_ge(sem, c * mult)
        nc.sync.drain()
        nc.all_engine_barrier()
        popped = nc._tile_sem_poison_stack.pop()
        assert popped is self._sem_poison
        nc.clear_and_free_semaphores(list(self.sems.allocated().values()))
        nc.all_engine_barrier()


# ---------------------------------------------------------------------------
# Walrus-compat post-pass: at most one sync wait per engine instruction
# (none on SP control ops). Hoist excess waits onto standalone
# InstEventSemaphore instructions inserted before.
# ---------------------------------------------------------------------------

_NO_HOIST = ("InstEventSemaphore", "InstAllEngineBarrier",
             "InstCollectiveCompute")


def _hoist_excess_waits(nc):
    n = 0
    for f in nc.m.functions:
        for bb in f.blocks:
            out = []
            changed = False
            for inst in bb.instructions:
                tname = type(inst).__name__
                si = inst.sync_info
                if si is not None and tname not in _NO_HOIST:
                    waits = list(si.on_wait)
                    limit = 0 if tname in ("InstDrain", "InstNoOp") else 1
                    if len(waits) > limit:
                        for w in waits[:len(waits) - limit]:
                            n += 1
                            ni = mybir.InstEventSemaphore(
                                name=f"I-hoist{n}", ins=[], outs=[])
                            ni.engine = inst.engine
                            ni.sync_info = mybir.SyncInfo(on_wait=[w],
                                                          on_update=[])
                            out.append(ni)
                        si.on_wait = waits[len(waits) - limit:]
                        changed = True
                out.append(inst)
            if changed:
                bb.instructions = out
    return n


# ---------------------------------------------------------------------------
# Values: SBUF tensor [128, NFC, TOK] plus a symbolic host scalar.
# true value = mult * tensor. unit => tensor is a unit LayerNorm output.
# ---------------------------------------------------------------------------

class Val:
    def __init__(self, t, mult=1.0, unit=False):
        self.t = t
        self.mult = float(mult)
        self.unit = unit


class Builder:
    def __init__(self, nc, tc, ctx):
        self.nc = nc
        self.tc = tc
        self.uploads = {}
        self.n_tag = 0
        self.pool = ctx.enter_context(tc.tile_pool(name="act", bufs=1))
        self.ps_pool = ctx.enter_context(
            tc.tile_pool(name="ps", bufs=4, space="PSUM"))
        self.ps_bf = ctx.enter_context(
            tc.tile_pool(name="psb", bufs=2, space="PSUM"))
        self.ps_stat = ctx.enter_context(
            tc.tile_pool(name="pstat", bufs=2, space="PSUM"))
        self.ident_bf = self.pool.tile([128, 128], BF16, tag="idb")
        ih = self.upload("ident", np.eye(128).astype(ml_dtypes.bfloat16),
                         [128, 128], BF16)
        nc.sync.dma_start(self.ident_bf[:, :], ih[:, :])
        self.ones_bf = self.pool.tile([128, 1], BF16, tag="ones")
        nc.vector.memset(self.ones_bf, 1.0)
        self.ones_row_bf = self.pool.tile([1, 128], BF16, tag="onesr")
        nc.vector.memset(self.ones_row_bf, 1.0)
        self._cc_cache = {}
        self.stats_cache = {}   # id(tensor) -> (rb_sb, mr_sb)
        self.ln_cache = {}      # id(tensor) -> Val (unit LN)
        # shared scratch (serial across stats/LN calls)
        self.sm_shared = self.pool.tile([1, 4 * TOK], F32, tag="smsh")
        self.rm_shared = self.pool.tile([1, 2 * TOK], BF16, tag="rmsh")
        self.x2_shared = self.pool.tile([128, NFC, TOK], BF16, tag="x2sh")
        self.lt_shared = self.pool.tile([128, NFC, TOK], BF16, tag="ltsh")
        self.rt_shared = self.pool.tile([128, NFC, TOK], BF16, tag="rtsh")

    def tag(self, kind="t"):
        self.n_tag += 1
        return f"{kind}{self.n_tag}"

    def sb(self, shape, dtype, kind="a"):
        tg = self.tag(kind)
        return self.pool.tile(list(shape), dtype, tag=tg, name=tg)

    def const_col(self, value, parts=1):
        key = (float(value), parts)
        if key not in self._cc_cache:
            t = self.pool.tile([parts, 1], F32, tag=self.tag("cc"))
            self.nc.vector.memset(t, float(value))
            self._cc_cache[key] = t
        return self._cc_cache[key]

    # -- host->device uploads -----------------------------------------------
    def upload(self, base, arrs, shape, dtype):
        name = f"{base}{len(self.uploads)}"
        if not isinstance(arrs, list):
            arrs = [arrs] * NCORE
        self.uploads[name] = [np.ascontiguousarray(a) for a in arrs]
        return self.nc.declare_dram_parameter(name, list(shape), dtype,
                                              isOutput=False)

    def upload_weight(self, w_np):
        """w_np [512, 512] (in, out) -> bf16 SBUF tile [128, NFC, 512]."""
        arr = np.ascontiguousarray(
            np.asarray(w_np, np.float32).reshape(NFC, 128, ISIZE)
            .transpose(1, 0, 2)).astype(ml_dtypes.bfloat16)
        hdl = self.upload("w", arr, [128, NFC, ISIZE], BF16)
        t = self.sb([128, NFC, ISIZE], BF16, kind="w")
        self.nc.sync.dma_start(t[:, :, :], hdl[:, :, :])
        return t

    def upload_bias(self, b_np):
        """b_np [512] -> SBUF [128, NFC] f32 (per-partition scalars)."""
        arr = np.ascontiguousarray(
            np.asarray(b_np, np.float32).reshape(NFC, 128).transpose(1, 0))
        hdl = self.upload("b", arr, [128, NFC], F32)
        t = self.sb([128, NFC], F32, kind="bias")
        self.nc.sync.dma_start(t[:, :], hdl[:, :])
        return t

    # -- input load ----------------------------------------------------------
    def load_input_fm(self, hdl):
        """DRAM [TOK, 512] bf16 token-major -> feature-major bf16 tensor."""
        nc = self.nc
        out = self.sb([128, NFC, TOK], BF16, kind="in")
        tok_tiles = []
        for tt in range(NTT):
            t = self.sb([128, ISIZE], BF16, kind="int")
            nc.sync.dma_start(t[:, :], hdl[ts(tt, 128), :])
            tok_tiles.append(t)
        for fc in range(NFC):
            ps = self.ps_bf.tile([128, TOK], BF16, tag="psb")
            for tt in range(NTT):
                nc.tensor.transpose(ps[:, ts(tt, 128)],
                                    tok_tiles[tt][:, ts(fc, 128)],
                                    self.ident_bf)
            if fc % 2 == 0:
                nc.scalar.activation(out[:, fc, :], ps[:, :], AF.Identity)
            else:
                nc.vector.tensor_copy(out[:, fc, :], ps[:, :])
        return Val(out, 1.0, False)

    # -- LayerNorm infra -----------------------------------------------------
    def ln_stats(self, val):
        """Per-token stats of the stored tensor: returns (rb_sb, mr_sb),
        both [128, TOK] bf16 broadcasts of rstd' and mean*rstd', such that
        LN(true) = tensor*rb - mr.  eps' = EPS / mult^2."""
        key = id(val.t)
        if key in self.stats_cache:
            return self.stats_cache[key]
        nc = self.nc
        x = val.t
        x2 = self.x2_shared
        m_ps = self.ps_stat.tile([1, TOK], F32, tag="st")
        s2_ps = self.ps_stat.tile([1, TOK], F32, tag="st")
        for kc in range(NFC):
            nc.tensor.matmul(m_ps[:, :], self.ones_bf[:, :], x[:, kc, :],
                             start=(kc == 0), stop=(kc == NFC - 1))
            nc.vector.tensor_mul(x2[:, kc, :], x[:, kc, :], x[:, kc, :])
            nc.tensor.matmul(s2_ps[:, :], self.ones_bf[:, :], x2[:, kc, :],
                             start=(kc == 0), stop=(kc == NFC - 1))
        # mean row (bf16, for the fused mean-correction matmul)
        m_bf = self.sb([1, TOK], BF16, kind="mb")
        nc.scalar.activation(m_bf[:, :], m_ps[:, :], AF.Identity,
                             scale=1.0 / ISIZE)
        sm = self.sm_shared
        sv = sm[:, 0:TOK]
        nc.vector.scalar_tensor_tensor(sv, m_bf[:, :], -1.0, m_bf[:, :],
                                       op0=ALU.mult, op1=ALU.mult)  # -mean^2
        nc.vector.scalar_tensor_tensor(sv, s2_ps[:, :], 1.0 / ISIZE, sv,
                                       op0=ALU.mult, op1=ALU.add)   # var
        epsp = EPS / (val.mult * val.mult)
        r_bf = self.rm_shared
        nc.scalar.activation(sv, sv, AF.Ln, bias=self.const_col(epsp))
        nc.scalar.activation(r_bf[:, 0:TOK], sv, AF.Exp, scale=-0.5)
        rb_ps = self.ps_pool.tile([128, TOK], F32, tag="ps")
        nc.tensor.matmul(rb_ps[:, :], self.ones_row_bf[:, :],
                         r_bf[:, 0:TOK], start=True, stop=True)
        rb_sb = self.sb([128, TOK], BF16, kind="rb")
        nc.scalar.activation(rb_sb[:, :], rb_ps[:, :], AF.Identity)
        ent = dict(m_bf=m_bf, rb=rb_sb, mr=None)
        self.stats_cache[key] = ent
        return ent

    def ln_mr(self, val):
        """mr broadcast (mean*rstd, [128,TOK] bf16) for materializing."""
        ent = self.ln_stats(val)
        if ent['mr'] is None:
            nc = self.nc
            r_bf = self.rm_shared
            nc.vector.scalar_tensor_tensor(
                r_bf[:, TOK:2 * TOK], ent['m_bf'][:, :], 1.0,
                r_bf[:, 0:TOK], op0=ALU.mult, op1=ALU.mult)
            mr_ps = self.ps_pool.tile([128, TOK], F32, tag="ps")
            nc.tensor.matmul(mr_ps[:, :], self.ones_row_bf[:, :],
                             r_bf[:, TOK:2 * TOK], start=True, stop=True)
            mr_sb = self.sb([128, TOK], BF16, kind="mr")
            nc.scalar.activation(mr_sb[:, :], mr_ps[:, :], AF.Identity)
            ent['mr'] = mr_sb
        return ent

    def ln_of(self, val):
        """Materialized unit-LN of val (cached). Per-chunk two-pass apply:
        u = x*rb - mr."""
        if val.unit:
            kappa = 1.0 / np.sqrt(1.0 + EPS / (val.mult * val.mult))
            return Val(val.t, kappa, True)
        key = id(val.t)
        if key in self.ln_cache:
            return self.ln_cache[key]
        nc = self.nc
        ent = self.ln_mr(val)
        rb_sb, mr_sb = ent['rb'], ent['mr']
        u = self.sb([128, NFC, TOK], BF16, kind="ln")
        tmp = self.lt_shared
        for fc in range(NFC):
            nc.vector.tensor_mul(tmp[:, fc, :], val.t[:, fc, :], rb_sb[:, :])
            nc.vector.scalar_tensor_tensor(
                u[:, fc, :], mr_sb[:, :], -1.0, tmp[:, fc, :],
                op0=ALU.mult, op1=ALU.add)
        out = Val(u, 1.0, True)
        self.ln_cache[key] = out
        return out

    # -- matmul --------------------------------------------------------------
    def mm_site_ln(self, val, w_np, epilogue):
        """Fused-LN matmul: LN(val) @ w, running on the RAW tensor.
        Mean is subtracted inside PSUM via a K=1 matmul with the negated
        column sums of w; rstd is applied in the epilogue, which receives
        (mc, ps, rb)."""
        nc = self.nc
        ent = self.ln_stats(val)
        wbf = np.asarray(w_np, np.float32).astype(ml_dtypes.bfloat16)
        wt = self.upload_weight(wbf)
        wcs = np.ascontiguousarray(
            -wbf.astype(np.float32).sum(axis=0)[None, :]
        ).astype(ml_dtypes.bfloat16)
        hw = self.upload("wc", wcs, [1, ISIZE], BF16)
        wcs_t = self.sb([1, ISIZE], BF16, kind="wc")
        nc.sync.dma_start(wcs_t[:, :], hw[:, :])
        x = val.t
        for mc in range(NFC):
            ps = self.ps_pool.tile([128, TOK], F32, tag="ps")
            for kc in range(NFC):
                nc.tensor.matmul(ps[:, :], wt[:, kc, ts(mc, 128)],
                                 x[:, kc, :], start=(kc == 0), stop=False)
            nc.tensor.matmul(ps[:, :], wcs_t[0:1, ts(mc, 128)],
                             ent['m_bf'][:, :], start=False, stop=True)
            epilogue(mc, ps, ent['rb'])

    def mm_site(self, parts, epilogue):
        """sum_i parts[i] @ W_i accumulated per output chunk; epilogue(mc, ps)
        consumes each chunk's PSUM. parts: list of (tensor, W_np) with all
        scalars folded into W host-side."""
        nc = self.nc
        wts = [self.upload_weight(w) for _, w in parts]
        for mc in range(NFC):
            ps = self.ps_pool.tile([128, TOK], F32, tag="ps")
            n = len(parts) * NFC
            i = 0
            for wt, (x, _) in zip(wts, parts):
                for kc in range(NFC):
                    nc.tensor.matmul(ps[:, :], wt[:, kc, ts(mc, 128)],
                                     x[:, kc, :], start=(i == 0),
                                     stop=(i == n - 1))
                    i += 1
            epilogue(mc, ps)

    def relu_ps(self, out_ap, ps_ap, scale):
        self.nc.scalar.activation(out_ap, ps_ap, AF.Relu,
                                  scale=float(scale))

    def copy_ps(self, out_ap, ps_ap):
        self.nc.scalar.activation(out_ap, ps_ap, AF.Identity)

    def act_epilogue(self, func, out, scale=1.0, bias_t=None):
        """Returns an epilogue writing func(scale*ps + bias) into out."""
        nc = self.nc

        def epi(mc, ps):
            bias_ap = bias_t[:, mc:mc + 1] if bias_t is not None else 0.0
            nc.scalar.activation(out[:, mc, :], ps[:, :], func,
                                 bias=bias_ap, scale=float(scale))
        return epi


# ---------------------------------------------------------------------------
# Graph emission
# ---------------------------------------------------------------------------

def _emit_graph(bld, np_in, routes):
    nc = bld.nc
    eW = np.asarray(np_in['edge_W'], np.float64)
    eb = np.asarray(np_in['edge_b'], np.float64)
    eg = np.asarray(np_in['edge_g'], np.float64)
    ebe = np.asarray(np_in['edge_beta'], np.float64)
    nW = np.asarray(np_in['node_W'], np.float64)
    nb = np.asarray(np_in['node_b'], np.float64)
    ng = np.asarray(np_in['node_g'], np.float64)
    nbe = np.asarray(np_in['node_beta'], np.float64)
    node_p = np.asarray(np_in['node_p'], np.float64)
    edge_p = np.asarray(np_in['edge_p'], np.float64)

    for r in routes:
        assert r['act'] != 0, "attention routing not supported in v2 kernel"

    # ---- route analysis ----------------------------------------------------
    # edge list: one entry per (consumer c, which) with selection scalar.
    # uses[(c, which)] = dict(src, e, op, s)
    uses = {}
    processed = set()
    used_src = set()
    for c, r in enumerate(routes):
        lind, nsrc = r['lind'], r['nsrc']
        ep = edge_p[:, lind:lind + nsrc, :].reshape(3, -1)
        for which, sel in (('q', r['q']), ('k', r['k']), ('v', r['v'])):
            if sel is None:
                continue
            se, op = sel // 5, sel % 5
            src = -2 if se == 0 else r['snode'] + se
            logits = ep[{'q': 0, 'k': 1, 'v': 2}[which]]
            first5 = (which == 'v' and r['vmode'] == 'first5')
            if first5:
                logits = logits[:5]
            mask = _qmask(nsrc) if which == 'q' else r['km']
            if first5:
                mask = None
            s = _selw_np(logits, mask, sel)
            uses[(c, which)] = dict(src=src, e=lind + se, op=op, s=s)
            processed.add(src)
            used_src.add(src)

    # which sources need LN (feed op<=2 edges)
    needs_ln = {u['src'] for u in uses.values() if u['op'] <= 2}
    aw = {c: float(_softmax_np(node_p[c] / TAU)[routes[c]['act']])
          for c in range(NNOD)}

    # final-sum membership: nodes never consumed as a source
    rem_nodes = [i for i in range(NNOD) if i not in processed]

    # ---- value bookkeeping -------------------------------------------------
    outs = {}          # node idx -> Val
    raw_of = {}        # node idx -> pre-LN raw Val (for fused-LN consumers)
    edge_h = {}        # e -> Val  (raw h of LN-edge or linear edge, unscaled)
    edge_emitted = set()

    # acc: the final sum, accumulated in-place, f32, true scale
    acc = bld.sb([128, NFC, TOK], BF16, kind="acc")
    acc_started = [False]

    def acc_add_ps(mc, ps, scale=1.0):
        """acc[:, mc, :] += scale * ps   (or initialize)."""
        if not acc_started[0]:
            nc.scalar.activation(acc[:, mc, :], ps[:, :], AF.Identity,
                                 scale=float(scale))
        else:
            nc.vector.scalar_tensor_tensor(
                acc[:, mc, :], ps[:, :], float(scale), acc[:, mc, :],
                op0=ALU.mult, op1=ALU.add)

    def acc_add_full(x, scale):
        """acc += scale * x (full tile, SBUF tensor)."""
        assert acc_started[0]
        for fc in range(NFC):
            nc.vector.scalar_tensor_tensor(
                acc[:, fc, :], x[:, fc, :], float(scale), acc[:, fc, :],
                op0=ALU.mult, op1=ALU.add)

    def acc_mark_started():
        acc_started[0] = True

    # does this (c, which) use feed the final accumulator directly?
    # -> node c is in rem AND its act combines terms additively for this slot
    def direct_to_acc(c, which):
        if c not in rem_nodes:
            return False
        a = routes[c]['act']
        # act6: q + k ; act5: q + gelu(k@W1+b1) (q slot only)
        # act4: q*sig(k) + v (v slot only)
        return (a == 6) or (a == 5 and which == 'q') or \
               (a == 4 and which == 'v')

    # multiplier applied to node c's term for `which` inside the final sum
    def acc_scale(c, which):
        return aw[c] * uses[(c, which)]['s']

    # ---- edge emission -----------------------------------------------------
    def w_eff(u):
        """Effective weight for an edge use (LN affine folded; for op3 the
        source mult is folded by the caller)."""
        e, op = u['e'], u['op']
        if op <= 2:
            return eg[e][:, None] * eW[e]
        return eW[e]

    def b_eff(u):
        e, op = u['e'], u['op']
        if op <= 2:
            return ebe[e] @ eW[e] + eb[e]
        return eb[e]

    def edge_input_ready(u):
        """(val, fused) the edge's matmul streams, or None if not ready.
        For LN edges on a non-unit source (or one with a recorded raw
        tensor), the matmul fuses the LN on the raw tensor."""
        src = u['src']
        if src not in outs:
            return None
        v = outs[src]
        if u['op'] in (0, 1, 2):
            if src in raw_of:
                return (raw_of[src], True)
            if not v.unit:
                return (v, True)
            return (bld.ln_of(v), False)
        return (v, False)

    def emit_edge(c, which, u):
        """Emit matmul + epilogue for one (consumer, which) use."""
        op, e, s = u['op'], u['e'], u['s']
        src_v = outs[u['src']]
        if op == 4:
            return  # identity: no work
        ready = edge_input_ready(u)
        assert ready is not None
        lnv, fused = ready
        x, xmult = lnv.t, lnv.mult
        if fused:
            xmult = 1.0  # LN of the raw tensor is unit by construction
        # collect all uses sharing this e (same consumer & src by construct)
        forms = [(w2, u2) for (c2, w2), u2 in uses.items()
                 if c2 == c and u2['e'] == e and u2['op'] != 4]
        key = e
        if key in edge_emitted:
            return
        edge_emitted.add(key)
        ops = {u2['op'] for _, u2 in forms}
        w = w_eff(u) * xmult
        b = b_eff(u)
        has_b = bool(np.any(b))

        if fused and has_b:
            fused = False
            lnv2 = bld.ln_of(lnv)
            x, xmult = lnv2.t, lnv2.mult
            w = w_eff(u) * xmult

        if ops == {0} and len(forms) == 1:
            # relu-only: fold consumer scalar (>0) through relu; the bias is
            # pre-scaled so Relu(cs*ps + cs*b) = cs*relu(ps + b)
            if direct_to_acc(c, which):
                cs = acc_scale(c, which)
                bias_t = bld.upload_bias(np.asarray(b) * cs) if has_b else None
                if fused:
                    # relu(rb*ps)*cs = rb>0 -> cs*rb*relu(ps)
                    def epi(mc, ps, rb, _cs=cs):
                        t = bld.rt_shared[:, mc, :]
                        nc.scalar.activation(t, ps[:, :], AF.Relu)
                        if acc_started[0]:
                            t2 = bld.lt_shared[:, mc, :]
                            nc.vector.scalar_tensor_tensor(
                                t2, t, float(_cs), rb[:, :],
                                op0=ALU.mult, op1=ALU.mult)
                            nc.vector.scalar_tensor_tensor(
                                acc[:, mc, :], t2, 1.0, acc[:, mc, :],
                                op0=ALU.mult, op1=ALU.add)
                        else:
                            nc.vector.scalar_tensor_tensor(
                                acc[:, mc, :], t, float(_cs), rb[:, :],
                                op0=ALU.mult, op1=ALU.mult)
                    bld.mm_site_ln(lnv, w, epi)
                else:
                    def epi(mc, ps, _cs=cs, _bt=bias_t):
                        if acc_started[0]:
                            t = bld.rt_shared[:, mc, :]
                            if _bt is None:
                                bld.relu_ps(t, ps[:, :], _cs)
                            else:
                                nc.scalar.activation(
                                    t, ps[:, :], AF.Relu, scale=float(_cs),
                                    bias=_bt[:, mc:mc + 1])
                            nc.vector.scalar_tensor_tensor(
                                acc[:, mc, :], t, 1.0, acc[:, mc, :],
                                op0=ALU.mult, op1=ALU.add)
                        elif _bt is None:
                            bld.relu_ps(acc[:, mc, :], ps[:, :], _cs)
                        else:
                            nc.scalar.activation(
                                acc[:, mc, :], ps[:, :], AF.Relu,
                                scale=float(_cs), bias=_bt[:, mc:mc + 1])
                    bld.mm_site([(x, w)], epi)
                if not acc_started[0]:
                    acc_mark_started()
                edge_h[e] = ('in_acc', None)
            else:
                cs = s
                out = bld.sb([128, NFC, TOK], BF16, kind="eh")
                if fused:
                    def epi(mc, ps, rb, _c=cs):
                        t = bld.rt_shared[:, mc, :]
                        nc.scalar.activation(t, ps[:, :], AF.Relu)
                        nc.vector.scalar_tensor_tensor(
                            out[:, mc, :], t, float(_c), rb[:, :],
                            op0=ALU.mult, op1=ALU.mult)
                    bld.mm_site_ln(lnv, w, epi)
                elif has_b:
                    bias_t = bld.upload_bias(np.asarray(b) * cs)
                    bld.mm_site([(x, w)],
                                bld.act_epilogue(AF.Relu, out, scale=cs,
                                                 bias_t=bias_t))
                else:
                    bld.mm_site([(x, w)], lambda mc, ps, _c=cs:
                                bld.relu_ps(out[:, mc, :], ps[:, :], _c))
                edge_h[e] = ('relu_scaled', Val(out, 1.0))
        elif ops <= {2, 3} and len(forms) == 1 and direct_to_acc(c, which) \
                and not has_b:
            # linear, single use, straight into the final sum from PSUM
            cs = acc_scale(c, which)
            if fused:
                def epi(mc, ps, rb, _cs=cs):
                    if acc_started[0]:
                        t = bld.rt_shared[:, mc, :]
                        nc.vector.scalar_tensor_tensor(
                            t, ps[:, :], float(_cs), rb[:, :],
                            op0=ALU.mult, op1=ALU.mult)
                        nc.vector.scalar_tensor_tensor(
                            acc[:, mc, :], t, 1.0, acc[:, mc, :],
                            op0=ALU.mult, op1=ALU.add)
                    else:
                        nc.vector.scalar_tensor_tensor(
                            acc[:, mc, :], ps[:, :], float(_cs), rb[:, :],
                            op0=ALU.mult, op1=ALU.mult)
                bld.mm_site_ln(lnv, w, epi)
            else:
                def epi(mc, ps, _cs=cs):
                    acc_add_ps(mc, ps, _cs)
                bld.mm_site([(x, w)], epi)
            if not acc_started[0]:
                acc_mark_started()
            edge_h[e] = ('in_acc', None)
        else:
            # general: materialize h, then any relu/gelu forms
            out = bld.sb([128, NFC, TOK], BF16, kind="eh")
            if fused:
                bld.mm_site_ln(lnv, w, lambda mc, ps, rb:
                               nc.vector.scalar_tensor_tensor(
                                   out[:, mc, :], ps[:, :], 1.0, rb[:, :],
                                   op0=ALU.mult, op1=ALU.mult))
            elif has_b:
                bias_t = bld.upload_bias(b)
                bld.mm_site([(x, w)],
                            bld.act_epilogue(AF.Identity, out,
                                             bias_t=bias_t))
            else:
                bld.mm_site([(x, w)], lambda mc, ps:
                            bld.copy_ps(out[:, mc, :], ps[:, :]))
            edge_h[e] = ('h', Val(out, 1.0))

    def edge_value(c, which):
        """Val for an emitted edge use (h-form resolved per op), with the
        selection scalar NOT yet applied (returned separately)."""
        u = uses[(c, which)]
        if u['op'] == 4:
            v = outs[u['src']]
            return Val(v.t, v.mult * u['s'], v.unit)
        kind, hv = edge_h[u['e']]
        if kind == 'in_acc':
            return None  # already folded into acc
        if kind == 'relu_scaled':
            return Val(hv.t, 1.0)  # scalar already folded
        # kind == 'h'
        if u['op'] in (2, 3):
            return Val(hv.t, u['s'])
        # relu/gelu on materialized h (shared-form edges); unscaled, the
        # selection scalar is returned in the Val
        fkey = (u['e'], u['op'])
        if fkey not in edge_h:
            out = bld.sb([128, NFC, TOK], BF16, kind="ef")
            func = AF.Relu if u['op'] == 0 else AF.Gelu_apprx_tanh
            for fc in range(NFC):
                nc.scalar.activation(out[:, fc, :], hv.t[:, fc, :], func)
            edge_h[fkey] = ('f', Val(out, 1.0))
        fv = edge_h[fkey][1]
        return Val(fv.t, u['s'])

    def prefetch():
        """Emit every not-yet-emitted edge whose input tensor is ready,
        in consumer-node order."""
        for c2 in range(NNOD):
            for which in ('q', 'k', 'v'):
                if (c2, which) not in uses:
                    continue
                u = uses[(c2, which)]
                if u['op'] == 4 or u['e'] in edge_emitted:
                    continue
                if edge_input_ready(u) is not None:
                    emit_edge(c2, which, u)

    # ---- inputs ------------------------------------------------------------
    for nm, idx in (('inpute', -2), ('inputo', -1)):
        if idx in used_src:
            hdl = bld.upload(
                nm,
                [np.ascontiguousarray(
                    np.asarray(np_in[nm]).reshape(-1, ISIZE)
                    [i * TOK:(i + 1) * TOK].astype(ml_dtypes.bfloat16))
                 for i in range(NCORE)],
                [TOK, ISIZE], BF16)
            outs[idx] = bld.load_input_fm(hdl)
            if idx in needs_ln:
                bld.ln_stats(outs[idx])

    # ---- node loop ---------------------------------------------------------
    for c, r in enumerate(routes):
        act = r['act']
        a = aw[c]
        in_rem = c in rem_nodes

        # make sure this node's own edges exist (normally via prefetch)
        for which in ('q', 'k', 'v'):
            if (c, which) in uses and uses[(c, which)]['op'] != 4 \
                    and uses[(c, which)]['e'] not in edge_emitted:
                emit_edge(c, which, uses[(c, which)])

        if act == 7:
            qv = edge_value(c, 'q')
            g, bta = ng[c], nbe[c]
            plain_aff = np.all(g == 1.0) and not np.any(bta)
            needs_tensor = (c in rem_nodes) or any(
                u2['src'] == c and u2['op'] in (3, 4)
                for u2 in uses.values())
            if plain_aff and not needs_tensor and not qv.unit:
                # LN consumed only by fused-LN edges: stats suffice
                raw_of[c] = Val(qv.t, qv.mult, False)
                bld.ln_stats(raw_of[c])
                outs[c] = Val(qv.t, qv.mult, False)
            elif plain_aff:
                ln = bld.ln_of(qv)
                outs[c] = Val(ln.t, ln.mult * a, True)
            else:
                sc = bld.upload_bias(a * ln.mult * g)
                bi = bld.upload_bias(a * bta)
                o = bld.sb([128, NFC, TOK], BF16, kind="n7")
                for fc in range(NFC):
                    nc.scalar.activation(o[:, fc, :], ln.t[:, fc, :],
                                         AF.Identity, scale=sc[:, fc:fc + 1],
                                         bias=bi[:, fc:fc + 1])
                outs[c] = Val(o, 1.0, False)

        elif act == 4:
            # q * sigmoid(k) + v
            u_q, u_k = uses[(c, 'q')], uses[(c, 'k')]
            vv = edge_value(c, 'v')
            shared_g = (u_q['e'] == u_k['e'] and u_q['op'] == 1
                        and u_k['op'] == 1 and vv is not None
                        and edge_h.get(u_q['e'], (None,))[0] == 'h')
            if shared_g:
                # per-chunk pipeline: gelu -> sigmoid -> mul -> combine
                hv = edge_h[u_q['e']][1]
                g = bld.sb([128, NFC, TOK], BF16, kind="g4")
                sg = bld.sb([128, NFC, TOK], BF16, kind="sg")
                m = bld.sb([128, NFC, TOK], BF16, kind="m4")
                o = bld.sb([128, NFC, TOK], BF16, kind="n4")
                edge_h[(u_q['e'], 1)] = ('f', Val(g, 1.0))
                for fc in range(NFC):
                    nc.scalar.activation(g[:, fc, :], hv.t[:, fc, :],
                                         AF.Gelu_apprx_tanh)
                for fc in range(NFC):
                    nc.scalar.activation(sg[:, fc, :], g[:, fc, :],
                                         AF.Sigmoid, scale=float(u_k['s']))
                    nc.vector.tensor_mul(m[:, fc, :], g[:, fc, :],
                                         sg[:, fc, :])
                    nc.vector.scalar_tensor_tensor(
                        o[:, fc, :], m[:, fc, :],
                        float(u_q['s'] / vv.mult), vv.t[:, fc, :],
                        op0=ALU.mult, op1=ALU.add)
                outs[c] = Val(o, a * vv.mult, False)
                if in_rem:
                    acc_add_full(o, a * vv.mult)
            else:
                qv = edge_value(c, 'q')
                kv = edge_value(c, 'k')
                sg = bld.sb([128, NFC, TOK], BF16, kind="sg")
                for fc in range(NFC):
                    nc.scalar.activation(sg[:, fc, :], kv.t[:, fc, :],
                                         AF.Sigmoid, scale=float(kv.mult))
                m = bld.sb([128, NFC, TOK], BF16, kind="m4")
                nc.vector.tensor_mul(m[:, :, :], qv.t[:, :, :], sg[:, :, :])
                if in_rem and vv is None:
                    acc_add_full(m, a * qv.mult)
                    outs[c] = None
                else:
                    o = bld.sb([128, NFC, TOK], BF16, kind="n4")
                    for fc in range(NFC):
                        nc.vector.scalar_tensor_tensor(
                            o[:, fc, :], m[:, fc, :],
                            float(qv.mult / vv.mult), vv.t[:, fc, :],
                            op0=ALU.mult, op1=ALU.add)
                    outs[c] = Val(o, a * vv.mult, False)
                    if in_rem:
                        acc_add_full(o, a * vv.mult)

        elif act == 6:
            # q + k: both either already in acc or added now
            for which in ('q', 'k'):
                u = uses[(c, which)]
                ev = edge_value(c, which)
                if ev is None:
                    continue  # folded into acc from PSUM
                if in_rem:
                    acc_add_full(ev.t, a * ev.mult)
                else:
                    raise NotImplementedError("act6 feeding another node")
            outs[c] = None

        elif act == 5:
            # q + gelu(k@W1 + b1)
            kv = edge_value(c, 'k')
            w1 = nW[c, 1] * kv.mult
            b1 = nb[c, 1]
            bias_t = bld.upload_bias(b1) if np.any(b1) else None
            if in_rem:
                g7 = bld.sb([128, NFC, TOK], BF16, kind="g5")
                def epi(mc, ps):
                    bias_ap = bias_t[:, mc:mc + 1] if bias_t is not None \
                        else 0.0
                    nc.scalar.activation(g7[:, mc, :], ps[:, :],
                                         AF.Gelu_apprx_tanh, bias=bias_ap)
                    nc.vector.scalar_tensor_tensor(
                        acc[:, mc, :], g7[:, mc, :], float(a),
                        acc[:, mc, :], op0=ALU.mult, op1=ALU.add)
                bld.mm_site([(kv.t, w1)], epi)
                qv = edge_value(c, 'q')
                if qv is not None:
                    acc_add_full(qv.t, a * qv.mult)
                outs[c] = None
            else:
                g7 = bld.sb([128, NFC, TOK], BF16, kind="g5")
                bld.mm_site([(kv.t, w1)],
                            bld.act_epilogue(AF.Gelu_apprx_tanh, g7,
                                             bias_t=bias_t))
                qv = edge_value(c, 'q')
                o = bld.sb([128, NFC, TOK], BF16, kind="n5")
                for fc in range(NFC):
                    nc.vector.scalar_tensor_tensor(
                        o[:, fc, :], qv.t[:, fc, :], float(qv.mult),
                        g7[:, fc, :], op0=ALU.mult, op1=ALU.add)
                outs[c] = Val(o, a, False)

        elif act == 3:
            # q + relu(q@W0 + k@W1 + v@W2)@W3 + b3
            qv = edge_value(c, 'q')
            kv = edge_value(c, 'k')
            vv = edge_value(c, 'v')
            inner = bld.sb([128, NFC, TOK], BF16, kind="i3")
            parts = [(qv.t, nW[c, 0] * qv.mult),
                     (kv.t, nW[c, 1] * kv.mult),
                     (vv.t, nW[c, 2] * vv.mult)]
            bld.mm_site(parts, bld.act_epilogue(AF.Relu, inner))
            b3 = nb[c, 3]
            o = bld.sb([128, NFC, TOK], BF16, kind="n3")

            def epi3(mc, ps):
                nc.vector.scalar_tensor_tensor(
                    o[:, mc, :], qv.t[:, mc, :], float(qv.mult), ps[:, :],
                    op0=ALU.mult, op1=ALU.add)
            if np.any(b3):
                bt3 = bld.upload_bias(b3)
                tmp3 = bld.sb([128, NFC, TOK], F32, kind="t3")
                def epi3b(mc, ps):
                    nc.scalar.activation(tmp3[:, mc, :], ps[:, :],
                                         AF.Identity,
                                         bias=bt3[:, mc:mc + 1])
                    nc.vector.scalar_tensor_tensor(
                        o[:, mc, :], qv.t[:, mc, :], float(qv.mult),
                        tmp3[:, mc, :], op0=ALU.mult, op1=ALU.add)
                bld.mm_site([(inner, nW[c, 3])], epi3b)
            else:
                bld.mm_site([(inner, nW[c, 3])], epi3)
            outs[c] = Val(o, a, False)
            if in_rem:
                acc_add_full(o, a)

        elif act == 1:
            # q + (gelu(q@W0+b0) * (k@W1+b1)) @ W3 + b3
            qv = edge_value(c, 'q')
            kv = edge_value(c, 'k')
            g = bld.sb([128, NFC, TOK], BF16, kind="g1")
            b0t = bld.upload_bias(nb[c, 0]) if np.any(nb[c, 0]) else None
            bld.mm_site([(qv.t, nW[c, 0] * qv.mult)],
                        bld.act_epilogue(AF.Gelu_apprx_tanh, g, bias_t=b0t))
            kk = bld.sb([128, NFC, TOK], BF16, kind="k1")
            b1t = bld.upload_bias(nb[c, 1]) if np.any(nb[c, 1]) else None
            bld.mm_site([(kv.t, nW[c, 1] * kv.mult)],
                        bld.act_epilogue(AF.Identity, kk, bias_t=b1t))
            p = bld.sb([128, NFC, TOK], BF16, kind="p1")
            nc.vector.tensor_mul(p[:, :, :], g[:, :, :], kk[:, :, :])
            o = bld.sb([128, NFC, TOK], BF16, kind="n1")
            b3 = nb[c, 3]
            if np.any(b3):
                bt3 = bld.upload_bias(b3)
                tmp1 = bld.sb([128, NFC, TOK], F32, kind="t1")
                def epi1b(mc, ps):
                    nc.scalar.activation(tmp1[:, mc, :], ps[:, :],
                                         AF.Identity, bias=bt3[:, mc:mc + 1])
                    nc.vector.scalar_tensor_tensor(
                        o[:, mc, :], qv.t[:, mc, :], float(qv.mult),
                        tmp1[:, mc, :], op0=ALU.mult, op1=ALU.add)
                bld.mm_site([(p, nW[c, 3])], epi1b)
            else:
                def epi1(mc, ps):
                    nc.vector.scalar_tensor_tensor(
                        o[:, mc, :], qv.t[:, mc, :], float(qv.mult),
                        ps[:, :], op0=ALU.mult, op1=ALU.add)
                bld.mm_site([(p, nW[c, 3])], epi1)
            outs[c] = Val(o, a, False)
            if in_rem:
                acc_add_full(o, a)

        elif act == 2:
            # LN(q + k + v) (+ affine)
            qv = edge_value(c, 'q')
            kv = edge_value(c, 'k')
            vv = edge_value(c, 'v')
            s1 = bld.sb([128, NFC, TOK], BF16, kind="s2a")
            for fc in range(NFC):
                nc.vector.scalar_tensor_tensor(
                    s1[:, fc, :], qv.t[:, fc, :],
                    float(qv.mult / kv.mult), kv.t[:, fc, :],
                    op0=ALU.mult, op1=ALU.add)
            s2t = bld.sb([128, NFC, TOK], BF16, kind="s2b")
            for fc in range(NFC):
                nc.vector.scalar_tensor_tensor(
                    s2t[:, fc, :], vv.t[:, fc, :],
                    float(vv.mult / kv.mult), s1[:, fc, :],
                    op0=ALU.mult, op1=ALU.add)
            sv = Val(s2t, kv.mult, False)
            ln = bld.ln_of(sv)
            outs[c] = Val(ln.t, a, True)
            if in_rem:
                acc_add_full(ln.t, a)

        else:
            raise NotImplementedError(f"act {act}")

        prefetch()
        if c in needs_ln and outs.get(c) is not None and not outs[c].unit:
            bld.ln_of(outs[c])
            prefetch()

    return Val(acc, 1.0, False)


def _emit_final(bld, acc, out_hdl, out_g, out_beta):
    """Transpose to token-major (bf16), per-token LN, DMA out."""
    nc = bld.nc
    xbf = acc.t
    epsp = EPS / (acc.mult * acc.mult)
    need_aff = not (np.all(out_g == 1.0) and not np.any(out_beta))
    if need_aff:
        gh = bld.upload("og", np.tile(np.asarray(out_g, np.float32),
                                      (128, 1)), [128, ISIZE], F32)
        bh = bld.upload("ob", np.tile(np.asarray(out_beta, np.float32),
                                      (128, 1)), [128, ISIZE], F32)
        gt = bld.sb([128, ISIZE], F32, kind="og")
        bt = bld.sb([128, ISIZE], F32, kind="ob")
        nc.sync.dma_start(gt[:, :], gh[:, :])
        nc.sync.dma_start(bt[:, :], bh[:, :])
    eps_col = bld.const_col(epsp, 128)
    fo_tiles = [bld.sb([128, ISIZE], F32, kind="fo") for _ in range(2)]
    for tt in range(NTT):
        ps = bld.ps_bf.tile([128, ISIZE], BF16, tag="psb")
        for fc in range(NFC):
            nc.tensor.transpose(ps[:, ts(fc, 128)], xbf[:, fc, ts(tt, 128)],
                                bld.ident_bf)
        sm = bld.sb([128, 9], F32, kind="fs")
        stats, mv, rstd = sm[:, 0:6], sm[:, 6:8], sm[:, 8:9]
        nc.vector.bn_stats(stats, ps[:, :])
        nc.vector.bn_aggr(mv, stats)
        nc.scalar.activation(rstd, mv[:, 1:2], AF.Ln, bias=eps_col)
        nc.scalar.activation(rstd, rstd, AF.Exp, scale=-0.5)
        ot = fo_tiles[tt % 2]
        nc.vector.tensor_scalar(ot[:, :], ps[:, :], mv[:, 0:1], rstd,
                                op0=ALU.subtract, op1=ALU.mult)
        if need_aff:
            nc.vector.tensor_mul(ot[:, :], ot[:, :], gt[:, :])
            nc.vector.tensor_add(ot[:, :], ot[:, :], bt[:, :])
        nc.sync.dma_start(out_hdl[ts(tt, 128), :], ot[:, :])


def _build_and_run(inputs, trace=False, **run_kwargs):
    np_in = {k: np.asarray(v) for k, v in inputs.items()}
    routes = _routing(np_in['node_p'], np_in['edge_p'])

    nc = bass.Bass(num_devices=NCORE)
    out_hdl = nc.declare_dram_parameter("out", [TOK, ISIZE], F32,
                                        isOutput=True)
    with FixedTileContext(nc) as tc:
        with ExitStack() as ctx:
            bld = Builder(nc, tc, ctx)
            acc = _emit_graph(bld, np_in, routes)
            _emit_final(bld, acc, out_hdl, np.asarray(np_in['out_g']),
                        np.asarray(np_in['out_beta']))
            uploads = bld.uploads
    _hoist_excess_waits(nc)
    in_maps = [{nm: arrs[i] for nm, arrs in uploads.items()}
               for i in range(NCORE)]
    res = run_bass_kernel_spmd(nc, in_maps, core_ids=list(range(NCORE)),
                               trace=trace, **run_kwargs)
    out = np.concatenate([res.results[i]['out'] for i in range(NCORE)], 0)
    return out.reshape(B, SLEN, ISIZE).astype(np.float32), res


def kernel(**inputs):
    out, _ = _build_and_run(inputs)
    return out
